# revision 29
# baseline (speedup 1.0000x reference)
"""Trainium2 Bass kernel for nn_HamiltonianDynamics.

Math: with q = state[:, :8], p = state[:, 8:], every MLP evaluation in the
reference operates on per-batch means of q/p. Adding a constant c to every
element of a [8,256,256] block shifts its mean by exactly c, so the whole
update collapses to per-batch stats:

  out = (state + off[b, half]) * scale
  off_q[b] = dt*gH[b,p]/Nq,  off_p[b] = -dt*gH[b,q]/Nq
  scale    = 1 - 0.1*err/(norm+1e-10)

Fully data-parallel SPMD, no collectives: each core owns 4 whole batches,
so the offsets (the only per-element-visible quantity) are exactly
computable locally. Approximations, each with bounded, documented error
far below the 2e-2 gate (the output error is dominated by bf16 I/O
quantization at ~1.7e-3 norm-relative):
  * I/O staged in bf16 (halves HBM traffic; keeps full relative precision
    on tiny elements unlike fp16 — wide exponent, no subnormal loss).
  * The three leapfrog gradient evaluations sit within O(dt*g/Nq) ~ 1e-7
    of the same point, so one backprop supplies both offsets (offset error
    ~1e-13 absolute, seven orders below the bf16 output ulp).
  * scale-1 is O(err/norm) ~ 1e-13, ten orders below bf16 resolution, so
    scale uses per-core unbiased estimates: local err mean; norm^2 from a
    2-tile sum-of-squares subsample (the off-dependent norm^2 correction
    terms, ~1e-11 relative, are dropped).

Pipeline per core (engine queues are in-order; emission order is tuned so
shadowable work never blocks the critical path):
  A. 17 chunked bf16 loads; per-chunk DVE sums via tensor_scalar+accum_out
     (bf16 4x mode) fold into PSUM means via 1/Nq-scaled ones-matmuls; ACT
     Square+accum sumsq on 2 tiles. The norm/sqrt/reciprocal path runs
     here too, pulling both ACT table loads off the critical path.
  B. After the last chunk: psum->sbuf means copy (q-half staged early),
     then one interleaved MLP forward+backward (casimir-old shadowed into
     its handoff gaps), fused junctions via scalar_tensor_tensor with the
     W3^T.W4 column host-folded, casimir-new, and a single matmul that
     fuses err reduction x recs scaling x partition broadcast (recs-filled
     stationary built during phase A).
  C. In-place bf16 transform y = x*scale + off (DVE 4x) + 17 chunked
     stores; the first store slice is 512 columns so it issues early.

Engine-AP constraint: compute-engine APs must start at partition 0, so all
per-batch row vectors are [1,nb] partition-0 rows and the 2-feature input
layers are two accumulated K=1 matmuls with [1,n] stationaries from a
single-descriptor row pack.
"""

import numpy as np
from ml_dtypes import bfloat16

NCORES = 8
B, CH, H, W = 32, 16, 256, 256
BPC = B // NCORES          # batches per core
NTILES = BPC * 2           # (batch, half) tiles per core
P = 128
FREE = (CH // 2) * H * W // P   # 4096
NQ = (CH // 2) * H * W          # 524288
NSSQ = 2                   # tiles subsampled for the norm estimate

# packed-weights layouts: tall [128, NW] (full-height tensors) and a row
# pack [1, NR] for the partition-0-only [1,n] stationaries — the row pack
# DMA is a single descriptor (~free) instead of n full-height columns
_COLS = {}
_RCOLS = {}


def _col_layout():
    c = 0
    def put(name, cols):
        nonlocal c
        _COLS[name] = (c, c + cols)
        c += cols
    put("b1", 1); put("w2", 128); put("b2", 1)
    put("w3", 64); put("b3", 1)
    put("w4", 1); put("w4n", 1); put("c2c", 1)
    put("w1t", 2); put("w2t", 128); put("w3t", 128)
    put("cb1", 1); put("cw2", 32); put("cb2", 1)
    put("cw3", 4); put("werr", 1)
    return c


def _row_layout():
    c = 0
    def put(name, cols):
        nonlocal c
        _RCOLS[name] = (c, c + cols)
        c += cols
    put("w1a", 128); put("w1b", 128)
    put("cw1a", 64); put("cw1b", 64)
    put("aux", 3)
    return c


NW = _col_layout()
NR = _row_layout()

_CACHE: dict = {}


def build_nc(ncores=NCORES, bpc=BPC, free=FREE):
    import concourse.bass as bass
    import concourse.bacc as bacc
    import concourse.tile as tile
    import concourse.mybir as mybir
    from contextlib import ExitStack

    f32 = mybir.dt.float32
    f16 = mybir.dt.bfloat16
    AL = mybir.AluOpType
    AF = mybir.ActivationFunctionType
    AX = mybir.AxisListType

    ntiles = bpc * 2
    nb = bpc
    nq = float(P * free)

    nc = bacc.Bacc("TRN2", target_bir_lowering=False, debug=False,
                   num_devices=ncores)

    x = nc.dram_tensor("x", [ntiles, P, free], f16, kind="ExternalInput").ap()
    w = nc.dram_tensor("w", [P, NW], f32, kind="ExternalInput").ap()
    wr = nc.dram_tensor("wr", [1, NR], f32, kind="ExternalInput").ap()
    y = nc.dram_tensor("y", [ntiles, P, free], f16, kind="ExternalOutput").ap()

    with tile.TileContext(nc) as tc, ExitStack() as ctx:
        xpool = ctx.enter_context(tc.tile_pool(name="xp", bufs=1))
        wpool = ctx.enter_context(tc.tile_pool(name="wp", bufs=1))
        scr = ctx.enter_context(tc.tile_pool(name="scr", bufs=2))
        ch = ctx.enter_context(tc.tile_pool(name="ch", bufs=2))
        keep = ctx.enter_context(tc.tile_pool(name="keep", bufs=1))
        psum = ctx.enter_context(tc.tile_pool(name="ps", bufs=4, space="PSUM"))
        pstat = ctx.enter_context(tc.tile_pool(name="pst", bufs=1, space="PSUM"))
        pcas = ctx.enter_context(tc.tile_pool(name="pcas", bufs=2, space="PSUM"))

        ones_col = wpool.tile([128, 1], f32)     # lhsT for partition sums
        nc.vector.memset(ones_col[:], 1.0)
        mcol = wpool.tile([128, 1], f32)         # lhsT folding the 1/Nq mean
        nc.vector.memset(mcol[:], 1.0 / nq)
        ones_bc = wpool.tile([1, 128], f32)      # lhsT for partition broadcast
        nc.vector.memset(ones_bc[:], 1.0)

        # ---- phase A: load shard + per-(batch,half) stats ----
        # Each tile loads as two half-chunks so the DVE sum accumulation
        # (tensor_scalar identity with accum_out, bf16 fast mode) trails the
        # DMA stream by only half a tile. The two halves' partition sums are
        # folded in PSUM via accumulated ones-matmuls. Sum-of-squares only on
        # the first NSSQ tiles via ACT Square+accum (norm estimate input).
        hf = free // 2
        part_ps = pstat.tile([1, ntiles], f32, tag="stat")
        part_ss = pstat.tile([1, NSSQ], f32, tag="sstat")
        xts = []
        for t in range(ntiles):
            xt = xpool.tile([P, free], f16, tag=f"x{t}")
            qf = hf // 2
            bounds = ([0, hf, 2 * hf] if t < ntiles - 1 else
                      [0, hf, 3 * qf, 3 * qf + 512, 4 * qf])
            st = keep.tile([128, len(bounds) - 1], f32, tag=f"st{t}")
            for c in range(len(bounds) - 1):
                sl = slice(bounds[c], bounds[c + 1])
                nc.sync.dma_start(xt[:, sl], x[t][:, sl])
                nc.vector.tensor_scalar(xt[:, sl], xt[:, sl], scalar1=1.0,
                                        scalar2=0.0, op0=AL.mult, op1=AL.add,
                                        accum_out=st[:, c:c + 1])
                # s-major column (h*nb+b): q-means land in cols 0:nb,
                # p-means in nb:2nb, so the chain reads contiguous views
                mc = (t % 2) * nb + t // 2
                nc.tensor.matmul(part_ps[0:1, mc:mc + 1], mcol[:],
                                 st[:, c:c + 1], start=(c == 0),
                                 stop=(c == len(bounds) - 2))
            if t < NSSQ:
                st2 = keep.tile([128, 1], f32, tag=f"ss{t}")
                sq = scr.tile([P, free], f16, tag=f"sq{t}")
                nc.scalar.activation(sq[:], xt[:], AF.Square,
                                     accum_out=st2[:, 0:1])
                nc.tensor.matmul(part_ss[0:1, t:t + 1], ones_col[:],
                                 st2[:, 0:1], start=True, stop=True)
            if t == ntiles - 2:
                # q-means complete (tiles 0,2,4,6 all landed by the time this
                # copy's dependency resolves): stage the early half
                m_sb = keep.tile([1, ntiles], f32)
                nc.vector.tensor_copy(m_sb[0:1, 0:nb], part_ps[0:1, 0:nb])
            if t == NSSQ:
                # ---- early norm/scale-denominator path (runs during the
                # load phase). norm^2 = (ncores*ntiles/NSSQ)*(ssq subsample);
                # the off-dependent correction terms (2*off*sum + Nq*off^2)
                # are ~1e-11 of norm^2 — far below the subsample's own
                # statistical accuracy — and are dropped. Doing the sqrt
                # here keeps the ACT sqrt-table load (1.3us) off the
                # critical path: the tanh-set reload it forces also lands
                # before the chain starts.
                rs = keep.tile([1, NSSQ], f32)
                nc.vector.tensor_copy(rs[:], part_ss[:])
                norm2 = keep.tile([1, 1], f32)
                nc.vector.tensor_tensor(norm2[:], rs[0:1, 0:1], rs[0:1, 1:2],
                                        op=AL.add)
                nc.vector.tensor_scalar(norm2[:], norm2[:],
                                        scalar1=float(ncores * ntiles) / NSSQ,
                                        scalar2=None, op0=AL.mult)
                nrm = keep.tile([1, 1], f32)
                nc.scalar.sqrt(nrm[:], norm2[:])
                den = keep.tile([1, 1], f32)
                nc.vector.tensor_scalar(den[:], nrm[:], scalar1=1e-10,
                                        scalar2=None, op0=AL.add)
                rec = keep.tile([1, 1], f32)
                nc.vector.reciprocal(rec[:], den[:])
                recs = keep.tile([1, 1], f32)
                nc.vector.tensor_scalar(recs[:], rec[:],
                                        scalar1=-0.1 / (4.0 * nb),
                                        scalar2=None, op0=AL.mult)
                # rall[32,128] = recs everywhere: used as the stationary of
                # the final err matmul so reduction x recs-scale x partition-
                # broadcast collapse into that one matmul
                rrow = keep.tile([1, 128], f32)
                nc.vector.tensor_scalar(rrow[:], ones_bc[:],
                                        scalar1=recs[0:1, 0:1], scalar2=None,
                                        op0=AL.mult)
                prall = pcas.tile([32, 128], f32, tag="cps")
                nc.tensor.matmul(prall[:], ones_bc[0:1, 0:32], rrow[:],
                                 start=True, stop=True)
                rall = keep.tile([32, 128], f32)
                nc.vector.tensor_copy(rall[:], prall[:])
                # dummy tanh on the sqrt result: pulls the tanh-set table
                # reload (1.3us, forced by the sqrt-set switch above) into
                # the load phase. The data dependency on nrm stops the
                # out-of-order window from hoisting it before the sqrt.
                dummy = keep.tile([1, 1], f32)
                nc.scalar.activation(dummy[:], nrm[:], AF.Tanh)
            xts.append(xt)

        # packed weights: the single-descriptor row pack first (~free),
        # then the tall pack
        wrt = wpool.tile([1, NR], f32)
        nc.sync.dma_start(wrt[:], wr)
        wt = wpool.tile([P, NW], f32)
        nc.sync.dma_start(wt[:], w)

        def wap(name):
            if name in _RCOLS:
                c0, c1 = _RCOLS[name]
                return wrt[0:1, c0:c1]
            c0, c1 = _COLS[name]
            rows = {"b3": 64, "w4": 64, "w4n": 64, "c2c": 128, "w3t": 64,
                    "cb1": 64, "cw2": 64, "cb2": 32, "cw3": 32,
                    "werr": 32}.get(name, 128)
            return wt[0:rows, c0:c1]

        # per-batch means, s-major: cols 0:nb = mq (copied early — complete
        # once the last h=0 tile lands), nb:2nb = mp
        nc.vector.tensor_copy(m_sb[0:1, nb:2 * nb],
                              part_ps[0:1, nb:2 * nb])
        mq = m_sb[0:1, 0:nb]
        mp = m_sb[0:1, nb:2 * nb]

        # ---- phase C: scalar chain (features on partitions, batch on free) --
        aux = wap("aux")
        aux1, aux2 = aux[0:1, 1:2], aux[0:1, 2:3]

        # The three leapfrog gradient evaluations sit within O(dt*g/Nq)
        # ~ 1e-7 of the same point, so g1 == g2 == g3 to ~1e-6 relative and
        # one backprop supplies both offset rows:
        #   offq = dt*g[p]/Nq, offp = -dt*g[q]/Nq
        # (the collapse changes the offsets by ~1e-13 absolute — seven
        # orders below the bf16 output ulp).
        #
        # The casimir-at-original-means evaluation (g2o) is hand-interleaved
        # into the gH forward: every engine queue is in-order, so each g2o
        # op is emitted right after the gH op it can shadow.
        p1 = psum.tile([128, nb], f32, tag="ps")
        nc.tensor.matmul(p1[:], wap("w1a"), mq, start=True, stop=False)
        nc.tensor.matmul(p1[:], wap("w1b"), mp, start=False, stop=True)
        cq1 = pcas.tile([64, nb], f32, tag="cps")
        nc.tensor.matmul(cq1[:], wap("cw1a"), mq, start=True, stop=False)
        nc.tensor.matmul(cq1[:], wap("cw1b"), mp, start=False, stop=True)
        h1 = ch.tile([128, nb], f32, tag="h1")
        nc.scalar.activation(h1[:], p1[:], AF.Tanh, bias=wap("b1"))
        cg1 = ch.tile([64, nb], f32, tag="cg1")
        nc.scalar.activation(cg1[:], cq1[:], AF.Tanh, bias=wap("cb1"))
        p2 = psum.tile([128, nb], f32, tag="ps")
        nc.tensor.matmul(p2[:], wap("w2"), h1[:], start=True, stop=True)
        cq2 = pcas.tile([32, nb], f32, tag="cps")
        nc.tensor.matmul(cq2[:], wap("cw2"), cg1[:], start=True, stop=True)
        h2 = ch.tile([128, nb], f32, tag="h2")
        nc.scalar.activation(h2[:], p2[:], AF.Tanh, bias=wap("b2"))
        g2o = ch.tile([32, nb], f32, tag="g2o")
        nc.scalar.activation(g2o[:], cq2[:], AF.Tanh, bias=wap("cb2"))
        p3 = psum.tile([64, nb], f32, tag="ps")
        nc.tensor.matmul(p3[:], wap("w3"), h2[:], start=True, stop=True)
        h3 = ch.tile([64, nb], f32, tag="h3")
        nc.scalar.activation(h3[:], p3[:], AF.Tanh, bias=wap("b3"))
        # backward (see gH docstring for the d3/c2 folding)
        u3 = ch.tile([64, nb], f32, tag="d3")
        nc.vector.scalar_tensor_tensor(u3[:], h3[:], wap("w4n"), h3[:],
                                       op0=AL.mult, op1=AL.mult)
        pd2 = psum.tile([128, nb], f32, tag="ps")
        nc.tensor.matmul(pd2[:], wap("w3t"), u3[:], start=True, stop=True)
        t2 = ch.tile([128, nb], f32, tag="t2")
        nc.vector.tensor_tensor(t2[:], h2[:], h2[:], op=AL.mult)
        nc.vector.tensor_scalar(t2[:], t2[:], scalar1=-1.0, scalar2=1.0,
                                op0=AL.mult, op1=AL.add)
        d2 = ch.tile([128, nb], f32, tag="d2")
        nc.vector.scalar_tensor_tensor(d2[:], pd2[:], wap("c2c"), t2[:],
                                       op0=AL.add, op1=AL.mult)
        pd1 = psum.tile([128, nb], f32, tag="ps")
        nc.tensor.matmul(pd1[:], wap("w2t"), d2[:], start=True, stop=True)
        t1 = ch.tile([128, nb], f32, tag="t1")
        nc.vector.tensor_tensor(t1[:], h1[:], h1[:], op=AL.mult)
        nc.vector.tensor_scalar(t1[:], t1[:], scalar1=-1.0, scalar2=1.0,
                                op0=AL.mult, op1=AL.add)
        d1 = ch.tile([128, nb], f32, tag="d1")
        nc.vector.tensor_tensor(d1[:], t1[:], pd1[:], op=AL.mult)
        w1t = wap("w1t")
        pgq = psum.tile([1, nb], f32, tag="ps")
        nc.tensor.matmul(pgq[:], w1t[:, 0:1], d1[:], start=True, stop=True)
        pgp = psum.tile([1, nb], f32, tag="ps")
        nc.tensor.matmul(pgp[:], w1t[:, 1:2], d1[:], start=True, stop=True)
        g2ow = ch.tile([32, nb], f32, tag="g2ow")
        nc.vector.tensor_scalar(g2ow[:], g2o[:], scalar1=wap("werr"),
                                scalar2=None, op0=AL.mult)
        # shifted means via fused (pg * aux) + m — one DVE op each on the
        # g2n critical path; the raw offsets and their partition broadcast
        # run in parallel (they only gate the transform, which also needs
        # scale — the slower path)
        mpn = keep.tile([1, nb], f32)
        nc.vector.scalar_tensor_tensor(mpn[:], pgq[:], aux2, mp, op0=AL.mult,
                                       op1=AL.add)
        mq3 = keep.tile([1, nb], f32)
        nc.vector.scalar_tensor_tensor(mq3[:], pgp[:], aux1, mq, op0=AL.mult,
                                       op1=AL.add)

        # casimir err estimate at the shifted means. mpn is computed first
        # and consumed by the first accumulated matmul so the PE starts half
        # a hop sooner; the offset broadcast below is emitted after these
        # matmuls because it has ~2us of slack before the transform needs it
        cq1n = pcas.tile([64, nb], f32, tag="cps")
        nc.tensor.matmul(cq1n[:], wap("cw1b"), mpn[:], start=True, stop=False)
        nc.tensor.matmul(cq1n[:], wap("cw1a"), mq3[:], start=False, stop=True)

        Bv = keep.tile([1, 2 * nb], f32)
        nc.vector.tensor_scalar(Bv[0:1, 0:nb], pgp[:], scalar1=aux1,
                                scalar2=None, op0=AL.mult)
        nc.vector.tensor_scalar(Bv[0:1, nb:2 * nb], pgq[:], scalar1=aux2,
                                scalar2=None, op0=AL.mult)
        poffb = psum.tile([128, 2 * nb], f32, tag="ps")
        nc.tensor.matmul(poffb[:], ones_bc[:], Bv[:], start=True, stop=True)
        offb = keep.tile([128, 2 * nb], f32)
        nc.vector.tensor_copy(offb[:], poffb[:])

        cg1n = ch.tile([64, nb], f32, tag="cg1n")
        nc.scalar.activation(cg1n[:], cq1n[:], AF.Tanh, bias=wap("cb1"))
        cq2n = pcas.tile([32, nb], f32, tag="cps")
        nc.tensor.matmul(cq2n[:], wap("cw2"), cg1n[:], start=True, stop=True)
        g2n = ch.tile([32, nb], f32, tag="g2n")
        nc.scalar.activation(g2n[:], cq2n[:], AF.Tanh, bias=wap("cb2"))

        # err tail: errsum = sum(werr[j]*(g2n - g2o)[j,b]) with
        # werr = cW3 @ ones4 folded on the host; g2o*werr precomputed off
        # the critical path, so one fused DVE op + one matmul remain
        dws = keep.tile([32, 1], f32)
        dwt = ch.tile([32, nb], f32, tag="dwt")
        nc.vector.scalar_tensor_tensor(dwt[:], g2n[:], wap("werr"), g2ow[:],
                                       op0=AL.mult, op1=AL.subtract,
                                       accum_out=dws[:, 0:1])
        # scale-1 = recs * errsum on every partition in one matmul (rall is
        # the recs-filled stationary); the +1 rides the psum->sbuf copy
        pscale = psum.tile([128, 1], f32, tag="ps")
        nc.tensor.matmul(pscale[:], rall[:], dws[:], start=True, stop=True)
        scb = keep.tile([128, 1], f32)
        nc.vector.tensor_scalar(scb[:], pscale[:], scalar1=1.0, scalar2=None,
                                op0=AL.add)

        # ---- phase E: in-place transform + store (half tiles so the first
        # store launches half a tile after scale lands) ----
        for t in range(ntiles):
            bl, h = t // 2, t % 2
            col = h * nb + bl
            xt = xts[t]
            bounds = [0, 512, hf, 2 * hf] if t == 0 else [0, hf, 2 * hf]
            for c in range(len(bounds) - 1):
                sl = slice(bounds[c], bounds[c + 1])
                # y = x*scale + off (the off term is applied unscaled:
                # off*(1-scale) ~ 1e-20 — utterly below any representable
                # difference)
                nc.vector.tensor_scalar(xt[:, sl], xt[:, sl],
                                        scalar1=scb[:, 0:1],
                                        scalar2=offb[:, col:col + 1],
                                        op0=AL.mult, op1=AL.add)
                nc.sync.dma_start(y[t][:, sl], xt[:, sl])

    nc.compile()
    return nc


def make_in_maps(inputs, ncores=NCORES, bpc=BPC, free=FREE):
    state = np.asarray(inputs["state"])
    dt = float(np.asarray(inputs["dt"]))
    nq = float(P * free)
    f = np.float32
    g = lambda k: np.ascontiguousarray(np.asarray(inputs[k], dtype=f))
    hW1, hW2, hW3, hW4 = g("hW1"), g("hW2"), g("hW3"), g("hW4")
    cW1 = g("cW1")

    wpack = np.zeros((P, NW), dtype=f)
    rpack = np.zeros((1, NR), dtype=f)
    def put(name, arr):
        c0, c1 = _COLS[name]
        arr = np.asarray(arr, dtype=f)
        wpack[:arr.shape[0], c0:c1] = arr
    def putr(name, vec):
        c0, c1 = _RCOLS[name]
        rpack[0, c0:c1] = np.asarray(vec, dtype=f).ravel()
    putr("w1a", hW1[0, :])
    putr("w1b", hW1[1, :])
    putr("cw1a", cW1[0, :])
    putr("cw1b", cW1[1, :])
    put("b1", g("hb1").reshape(128, 1))
    put("w2", hW2)
    put("b2", g("hb2").reshape(128, 1))
    put("w3", hW3)
    put("b3", g("hb3").reshape(64, 1))
    put("w4", hW4.reshape(64, 1))
    put("w4n", -hW4.reshape(64, 1))
    put("c2c", (hW3 @ hW4).reshape(128, 1))
    put("w1t", hW1.T)
    put("w2t", hW2.T)
    put("w3t", hW3.T)
    put("cb1", g("cb1").reshape(64, 1))
    put("cw2", g("cW2"))
    put("cb2", g("cb2").reshape(32, 1))
    put("cw3", g("cW3"))
    put("werr", g("cW3") @ np.ones((4, 1), dtype=f))
    rpack[0, _RCOLS["aux"][0]] = -0.5 * dt / nq
    rpack[0, _RCOLS["aux"][0] + 1] = dt / nq
    rpack[0, _RCOLS["aux"][0] + 2] = -dt / nq

    in_maps = []
    for i in range(ncores):
        shard = state[i * bpc:(i + 1) * bpc].astype(bfloat16).reshape(
            2 * bpc, P, free)
        in_maps.append({"x": shard, "w": wpack, "wr": rpack})
    return in_maps


def kernel(**inputs):
    from concourse.bass_utils import run_bass_kernel_spmd

    if "nc" not in _CACHE:
        _CACHE["nc"] = build_nc()
    nc = _CACHE["nc"]
    in_maps = make_in_maps(inputs)
    res = run_bass_kernel_spmd(nc, in_maps, list(range(NCORES)))
    out = np.concatenate(
        [res.results[i]["y"].astype(np.float32).reshape(BPC, CH, H, W)
         for i in range(NCORES)],
        axis=0)
    return out


# revision 30
# speedup vs baseline: 1.0027x; 1.0027x over previous
"""Trainium2 Bass kernel for nn_HamiltonianDynamics.

Math: with q = state[:, :8], p = state[:, 8:], every MLP evaluation in the
reference operates on per-batch means of q/p. Adding a constant c to every
element of a [8,256,256] block shifts its mean by exactly c, so the whole
update collapses to per-batch stats:

  out = (state + off[b, half]) * scale
  off_q[b] = dt*gH[b,p]/Nq,  off_p[b] = -dt*gH[b,q]/Nq
  scale    = 1 - 0.1*err/(norm+1e-10)

Fully data-parallel SPMD, no collectives: each core owns 4 whole batches,
so the offsets (the only per-element-visible quantity) are exactly
computable locally. Approximations, each with bounded, documented error
far below the 2e-2 gate (the output error is dominated by bf16 I/O
quantization at ~1.7e-3 norm-relative):
  * I/O staged in bf16 (halves HBM traffic; keeps full relative precision
    on tiny elements unlike fp16 — wide exponent, no subnormal loss).
  * The three leapfrog gradient evaluations sit within O(dt*g/Nq) ~ 1e-7
    of the same point, so one backprop supplies both offsets (offset error
    ~1e-13 absolute, seven orders below the bf16 output ulp).
  * scale-1 is O(err/norm) ~ 1e-13, ten orders below bf16 resolution, so
    scale uses per-core unbiased estimates: local err mean; norm^2 from a
    2-tile sum-of-squares subsample (the off-dependent norm^2 correction
    terms, ~1e-11 relative, are dropped).

Pipeline per core (engine queues are in-order; emission order is tuned so
shadowable work never blocks the critical path):
  A. 17 chunked bf16 loads; per-chunk DVE sums via tensor_scalar+accum_out
     (bf16 4x mode) fold into PSUM means via 1/Nq-scaled ones-matmuls; ACT
     Square+accum sumsq on 2 tiles. The norm/sqrt/reciprocal path runs
     here too, pulling both ACT table loads off the critical path.
  B. After the last chunk: psum->sbuf means copy (q-half staged early),
     then one interleaved MLP forward+backward (casimir-old shadowed into
     its handoff gaps), fused junctions via scalar_tensor_tensor with the
     W3^T.W4 column host-folded, casimir-new, and a single matmul that
     fuses err reduction x recs scaling x partition broadcast (recs-filled
     stationary built during phase A).
  C. In-place bf16 transform y = x*scale + off (DVE 4x) + 17 chunked
     stores; the first store slice is 512 columns so it issues early.

Engine-AP constraint: compute-engine APs must start at partition 0, so all
per-batch row vectors are [1,nb] partition-0 rows and the 2-feature input
layers are two accumulated K=1 matmuls with [1,n] stationaries from a
single-descriptor row pack.
"""

import numpy as np
from ml_dtypes import bfloat16

NCORES = 8
B, CH, H, W = 32, 16, 256, 256
BPC = B // NCORES          # batches per core
NTILES = BPC * 2           # (batch, half) tiles per core
P = 128
FREE = (CH // 2) * H * W // P   # 4096
NQ = (CH // 2) * H * W          # 524288
NSSQ = 2                   # tiles subsampled for the norm estimate

# packed-weights layouts: tall [128, NW] (full-height tensors) and a row
# pack [1, NR] for the partition-0-only [1,n] stationaries — the row pack
# DMA is a single descriptor (~free) instead of n full-height columns
_COLS = {}
_RCOLS = {}


def _col_layout():
    c = 0
    def put(name, cols):
        nonlocal c
        _COLS[name] = (c, c + cols)
        c += cols
    put("b1", 1); put("w2", 128); put("b2", 1)
    put("w3", 64); put("b3", 1)
    put("w4", 1); put("w4n", 1); put("c2c", 1)
    put("w1t", 2); put("w2t", 128); put("w3t", 128)
    put("cb1", 1); put("cw2", 32); put("cb2", 1)
    put("cw3", 4); put("werr", 1)
    return c


def _row_layout():
    c = 0
    def put(name, cols):
        nonlocal c
        _RCOLS[name] = (c, c + cols)
        c += cols
    put("w1a", 128); put("w1b", 128)
    put("cw1a", 64); put("cw1b", 64)
    put("aux", 5)
    return c


NW = _col_layout()
NR = _row_layout()

_CACHE: dict = {}


def build_nc(ncores=NCORES, bpc=BPC, free=FREE):
    import concourse.bass as bass
    import concourse.bacc as bacc
    import concourse.tile as tile
    import concourse.mybir as mybir
    from contextlib import ExitStack

    f32 = mybir.dt.float32
    f16 = mybir.dt.bfloat16
    AL = mybir.AluOpType
    AF = mybir.ActivationFunctionType
    AX = mybir.AxisListType

    ntiles = bpc * 2
    nb = bpc
    nq = float(P * free)

    nc = bacc.Bacc("TRN2", target_bir_lowering=False, debug=False,
                   num_devices=ncores)

    x = nc.dram_tensor("x", [ntiles, P, free], f16, kind="ExternalInput").ap()
    w = nc.dram_tensor("w", [P, NW], f32, kind="ExternalInput").ap()
    wr = nc.dram_tensor("wr", [1, NR], f32, kind="ExternalInput").ap()
    y = nc.dram_tensor("y", [ntiles, P, free], f16, kind="ExternalOutput").ap()

    with tile.TileContext(nc) as tc, ExitStack() as ctx:
        xpool = ctx.enter_context(tc.tile_pool(name="xp", bufs=1))
        wpool = ctx.enter_context(tc.tile_pool(name="wp", bufs=1))
        scr = ctx.enter_context(tc.tile_pool(name="scr", bufs=2))
        ch = ctx.enter_context(tc.tile_pool(name="ch", bufs=2))
        keep = ctx.enter_context(tc.tile_pool(name="keep", bufs=1))
        psum = ctx.enter_context(tc.tile_pool(name="ps", bufs=4, space="PSUM"))
        pstat = ctx.enter_context(tc.tile_pool(name="pst", bufs=1, space="PSUM"))
        pcas = ctx.enter_context(tc.tile_pool(name="pcas", bufs=2, space="PSUM"))

        ones_col = wpool.tile([128, 1], f32)     # lhsT for partition sums
        nc.vector.memset(ones_col[:], 1.0)
        ones_bc = wpool.tile([1, 128], f32)      # lhsT for partition broadcast
        nc.vector.memset(ones_bc[:], 1.0)

        # ---- phase A: load shard + per-(batch,half) stats ----
        # Each tile loads as two half-chunks so the DVE sum accumulation
        # (tensor_scalar identity with accum_out, bf16 fast mode) trails the
        # DMA stream by only half a tile. The two halves' partition sums are
        # folded in PSUM via accumulated ones-matmuls. Sum-of-squares only on
        # the first NSSQ tiles via ACT Square+accum (norm estimate input).
        hf = free // 2
        part_ss = pstat.tile([1, NSSQ], f32, tag="sstat")
        # raw per-batch sums, s-major (cols 0:nb = q, nb:2nb = p), written
        # directly by per-tile Pool cross-partition reduces
        m_sb = keep.tile([1, ntiles], f32)
        xts = []
        for t in range(ntiles):
            xt = xpool.tile([P, free], f16, tag=f"x{t}")
            qf = hf // 2
            bounds = ([0, hf, 2 * hf] if t < ntiles - 1 else
                      [0, hf, 3 * qf, 3 * qf + 512, 4 * qf])
            st = keep.tile([128, len(bounds) - 1], f32, tag=f"st{t}")
            for c in range(len(bounds) - 1):
                sl = slice(bounds[c], bounds[c + 1])
                nc.sync.dma_start(xt[:, sl], x[t][:, sl])
                nc.vector.tensor_scalar(xt[:, sl], xt[:, sl], scalar1=1.0,
                                        scalar2=0.0, op0=AL.mult, op1=AL.add,
                                        accum_out=st[:, c:c + 1])
            # tile total via a Pool cross-partition reduce straight into the
            # sums row — no PE matmul, no PSUM->SBUF hop on the tail
            mc = (t % 2) * nb + t // 2
            nc.gpsimd.tensor_reduce(m_sb[0:1, mc:mc + 1], st[:],
                                    axis=AX.XYZWC, op=AL.add)
            if t < NSSQ:
                st2 = keep.tile([128, 1], f32, tag=f"ss{t}")
                sq = scr.tile([P, free], f16, tag=f"sq{t}")
                nc.scalar.activation(sq[:], xt[:], AF.Square,
                                     accum_out=st2[:, 0:1])
                nc.tensor.matmul(part_ss[0:1, t:t + 1], ones_col[:],
                                 st2[:, 0:1], start=True, stop=True)
            if t == NSSQ:
                # ---- early norm/scale-denominator path (runs during the
                # load phase). norm^2 = (ncores*ntiles/NSSQ)*(ssq subsample);
                # the off-dependent correction terms (2*off*sum + Nq*off^2)
                # are ~1e-11 of norm^2 — far below the subsample's own
                # statistical accuracy — and are dropped. Doing the sqrt
                # here keeps the ACT sqrt-table load (1.3us) off the
                # critical path: the tanh-set reload it forces also lands
                # before the chain starts.
                rs = keep.tile([1, NSSQ], f32)
                nc.vector.tensor_copy(rs[:], part_ss[:])
                norm2 = keep.tile([1, 1], f32)
                nc.vector.tensor_tensor(norm2[:], rs[0:1, 0:1], rs[0:1, 1:2],
                                        op=AL.add)
                nc.vector.tensor_scalar(norm2[:], norm2[:],
                                        scalar1=float(ncores * ntiles) / NSSQ,
                                        scalar2=None, op0=AL.mult)
                nrm = keep.tile([1, 1], f32)
                nc.scalar.sqrt(nrm[:], norm2[:])
                den = keep.tile([1, 1], f32)
                nc.vector.tensor_scalar(den[:], nrm[:], scalar1=1e-10,
                                        scalar2=None, op0=AL.add)
                rec = keep.tile([1, 1], f32)
                nc.vector.reciprocal(rec[:], den[:])
                recs = keep.tile([1, 1], f32)
                nc.vector.tensor_scalar(recs[:], rec[:],
                                        scalar1=-0.1 / (4.0 * nb),
                                        scalar2=None, op0=AL.mult)
                # rall[32,128] = recs everywhere: used as the stationary of
                # the final err matmul so reduction x recs-scale x partition-
                # broadcast collapse into that one matmul
                rrow = keep.tile([1, 128], f32)
                nc.vector.tensor_scalar(rrow[:], ones_bc[:],
                                        scalar1=recs[0:1, 0:1], scalar2=None,
                                        op0=AL.mult)
                prall = pcas.tile([32, 128], f32, tag="cps")
                nc.tensor.matmul(prall[:], ones_bc[0:1, 0:32], rrow[:],
                                 start=True, stop=True)
                rall = keep.tile([32, 128], f32)
                nc.vector.tensor_copy(rall[:], prall[:])
                # dummy tanh on the sqrt result: pulls the tanh-set table
                # reload (1.3us, forced by the sqrt-set switch above) into
                # the load phase. The data dependency on nrm stops the
                # out-of-order window from hoisting it before the sqrt.
                dummy = keep.tile([1, 1], f32)
                nc.scalar.activation(dummy[:], nrm[:], AF.Tanh)
            xts.append(xt)

        # packed weights: the single-descriptor row pack first (~free),
        # then the tall pack
        wrt = wpool.tile([1, NR], f32)
        nc.sync.dma_start(wrt[:], wr)
        wt = wpool.tile([P, NW], f32)
        nc.sync.dma_start(wt[:], w)

        def wap(name):
            if name in _RCOLS:
                c0, c1 = _RCOLS[name]
                return wrt[0:1, c0:c1]
            c0, c1 = _COLS[name]
            rows = {"b3": 64, "w4": 64, "w4n": 64, "c2c": 128, "w3t": 64,
                    "cb1": 64, "cw2": 64, "cb2": 32, "cw3": 32,
                    "werr": 32}.get(name, 128)
            return wt[0:rows, c0:c1]

        # raw per-batch sums, s-major (the 1/Nq lives in the layer-1
        # stationaries and the aux constants)
        mq = m_sb[0:1, 0:nb]
        mp = m_sb[0:1, nb:2 * nb]

        # ---- phase C: scalar chain (features on partitions, batch on free) --
        aux = wap("aux")
        aux1, aux2 = aux[0:1, 1:2], aux[0:1, 2:3]
        aux3, aux4 = aux[0:1, 3:4], aux[0:1, 4:5]

        # The three leapfrog gradient evaluations sit within O(dt*g/Nq)
        # ~ 1e-7 of the same point, so g1 == g2 == g3 to ~1e-6 relative and
        # one backprop supplies both offset rows:
        #   offq = dt*g[p]/Nq, offp = -dt*g[q]/Nq
        # (the collapse changes the offsets by ~1e-13 absolute — seven
        # orders below the bf16 output ulp).
        #
        # The casimir-at-original-means evaluation (g2o) is hand-interleaved
        # into the gH forward: every engine queue is in-order, so each g2o
        # op is emitted right after the gH op it can shadow.
        p1 = psum.tile([128, nb], f32, tag="ps")
        nc.tensor.matmul(p1[:], wap("w1a"), mq, start=True, stop=False)
        nc.tensor.matmul(p1[:], wap("w1b"), mp, start=False, stop=True)
        cq1 = pcas.tile([64, nb], f32, tag="cps")
        nc.tensor.matmul(cq1[:], wap("cw1a"), mq, start=True, stop=False)
        nc.tensor.matmul(cq1[:], wap("cw1b"), mp, start=False, stop=True)
        h1 = ch.tile([128, nb], f32, tag="h1")
        nc.scalar.activation(h1[:], p1[:], AF.Tanh, bias=wap("b1"))
        cg1 = ch.tile([64, nb], f32, tag="cg1")
        nc.scalar.activation(cg1[:], cq1[:], AF.Tanh, bias=wap("cb1"))
        p2 = psum.tile([128, nb], f32, tag="ps")
        nc.tensor.matmul(p2[:], wap("w2"), h1[:], start=True, stop=True)
        cq2 = pcas.tile([32, nb], f32, tag="cps")
        nc.tensor.matmul(cq2[:], wap("cw2"), cg1[:], start=True, stop=True)
        h2 = ch.tile([128, nb], f32, tag="h2")
        nc.scalar.activation(h2[:], p2[:], AF.Tanh, bias=wap("b2"))
        g2o = ch.tile([32, nb], f32, tag="g2o")
        nc.scalar.activation(g2o[:], cq2[:], AF.Tanh, bias=wap("cb2"))
        p3 = psum.tile([64, nb], f32, tag="ps")
        nc.tensor.matmul(p3[:], wap("w3"), h2[:], start=True, stop=True)
        h3 = ch.tile([64, nb], f32, tag="h3")
        nc.scalar.activation(h3[:], p3[:], AF.Tanh, bias=wap("b3"))
        # backward (see gH docstring for the d3/c2 folding)
        u3 = ch.tile([64, nb], f32, tag="d3")
        nc.vector.scalar_tensor_tensor(u3[:], h3[:], wap("w4n"), h3[:],
                                       op0=AL.mult, op1=AL.mult)
        pd2 = psum.tile([128, nb], f32, tag="ps")
        nc.tensor.matmul(pd2[:], wap("w3t"), u3[:], start=True, stop=True)
        t2 = ch.tile([128, nb], f32, tag="t2")
        nc.vector.tensor_tensor(t2[:], h2[:], h2[:], op=AL.mult)
        nc.vector.tensor_scalar(t2[:], t2[:], scalar1=-1.0, scalar2=1.0,
                                op0=AL.mult, op1=AL.add)
        d2 = ch.tile([128, nb], f32, tag="d2")
        nc.vector.scalar_tensor_tensor(d2[:], pd2[:], wap("c2c"), t2[:],
                                       op0=AL.add, op1=AL.mult)
        pd1 = psum.tile([128, nb], f32, tag="ps")
        nc.tensor.matmul(pd1[:], wap("w2t"), d2[:], start=True, stop=True)
        t1 = ch.tile([128, nb], f32, tag="t1")
        nc.vector.tensor_tensor(t1[:], h1[:], h1[:], op=AL.mult)
        nc.vector.tensor_scalar(t1[:], t1[:], scalar1=-1.0, scalar2=1.0,
                                op0=AL.mult, op1=AL.add)
        d1 = ch.tile([128, nb], f32, tag="d1")
        nc.vector.tensor_tensor(d1[:], t1[:], pd1[:], op=AL.mult)
        w1t = wap("w1t")
        pgq = psum.tile([1, nb], f32, tag="ps")
        nc.tensor.matmul(pgq[:], w1t[:, 0:1], d1[:], start=True, stop=True)
        pgp = psum.tile([1, nb], f32, tag="ps")
        nc.tensor.matmul(pgp[:], w1t[:, 1:2], d1[:], start=True, stop=True)
        g2ow = ch.tile([32, nb], f32, tag="g2ow")
        nc.vector.tensor_scalar(g2ow[:], g2o[:], scalar1=wap("werr"),
                                scalar2=None, op0=AL.mult)
        # shifted means via fused (pg * aux) + m — one DVE op each on the
        # g2n critical path; the raw offsets and their partition broadcast
        # run in parallel (they only gate the transform, which also needs
        # scale — the slower path)
        mpn = keep.tile([1, nb], f32)
        nc.vector.scalar_tensor_tensor(mpn[:], pgq[:], aux4, mp, op0=AL.mult,
                                       op1=AL.add)
        mq3 = keep.tile([1, nb], f32)
        nc.vector.scalar_tensor_tensor(mq3[:], pgp[:], aux3, mq, op0=AL.mult,
                                       op1=AL.add)

        # casimir err estimate at the shifted means. mpn is computed first
        # and consumed by the first accumulated matmul so the PE starts half
        # a hop sooner; the offset broadcast below is emitted after these
        # matmuls because it has ~2us of slack before the transform needs it
        cq1n = pcas.tile([64, nb], f32, tag="cps")
        nc.tensor.matmul(cq1n[:], wap("cw1b"), mpn[:], start=True, stop=False)
        nc.tensor.matmul(cq1n[:], wap("cw1a"), mq3[:], start=False, stop=True)

        Bv = keep.tile([1, 2 * nb], f32)
        nc.vector.tensor_scalar(Bv[0:1, 0:nb], pgp[:], scalar1=aux1,
                                scalar2=None, op0=AL.mult)
        nc.vector.tensor_scalar(Bv[0:1, nb:2 * nb], pgq[:], scalar1=aux2,
                                scalar2=None, op0=AL.mult)
        poffb = psum.tile([128, 2 * nb], f32, tag="ps")
        nc.tensor.matmul(poffb[:], ones_bc[:], Bv[:], start=True, stop=True)
        offb = keep.tile([128, 2 * nb], f32)
        nc.vector.tensor_copy(offb[:], poffb[:])

        cg1n = ch.tile([64, nb], f32, tag="cg1n")
        nc.scalar.activation(cg1n[:], cq1n[:], AF.Tanh, bias=wap("cb1"))
        cq2n = pcas.tile([32, nb], f32, tag="cps")
        nc.tensor.matmul(cq2n[:], wap("cw2"), cg1n[:], start=True, stop=True)
        g2n = ch.tile([32, nb], f32, tag="g2n")
        nc.scalar.activation(g2n[:], cq2n[:], AF.Tanh, bias=wap("cb2"))

        # err tail: errsum = sum(werr[j]*(g2n - g2o)[j,b]) with
        # werr = cW3 @ ones4 folded on the host; g2o*werr precomputed off
        # the critical path, so one fused DVE op + one matmul remain
        dws = keep.tile([32, 1], f32)
        dwt = ch.tile([32, nb], f32, tag="dwt")
        nc.vector.scalar_tensor_tensor(dwt[:], g2n[:], wap("werr"), g2ow[:],
                                       op0=AL.mult, op1=AL.subtract,
                                       accum_out=dws[:, 0:1])
        # scale-1 = recs * errsum on every partition in one matmul (rall is
        # the recs-filled stationary); the +1 rides the psum->sbuf copy
        pscale = psum.tile([128, 1], f32, tag="ps")
        nc.tensor.matmul(pscale[:], rall[:], dws[:], start=True, stop=True)
        scb = keep.tile([128, 1], f32)
        nc.vector.tensor_scalar(scb[:], pscale[:], scalar1=1.0, scalar2=None,
                                op0=AL.add)

        # ---- phase E: in-place transform + store (half tiles so the first
        # store launches half a tile after scale lands) ----
        for t in range(ntiles):
            bl, h = t // 2, t % 2
            col = h * nb + bl
            xt = xts[t]
            bounds = [0, 512, hf, 2 * hf] if t == 0 else [0, hf, 2 * hf]
            for c in range(len(bounds) - 1):
                sl = slice(bounds[c], bounds[c + 1])
                # y = x*scale + off (the off term is applied unscaled:
                # off*(1-scale) ~ 1e-20 — utterly below any representable
                # difference)
                nc.vector.tensor_scalar(xt[:, sl], xt[:, sl],
                                        scalar1=scb[:, 0:1],
                                        scalar2=offb[:, col:col + 1],
                                        op0=AL.mult, op1=AL.add)
                nc.sync.dma_start(y[t][:, sl], xt[:, sl])

    nc.compile()
    return nc


def make_in_maps(inputs, ncores=NCORES, bpc=BPC, free=FREE):
    state = np.asarray(inputs["state"])
    dt = float(np.asarray(inputs["dt"]))
    nq = float(P * free)
    f = np.float32
    g = lambda k: np.ascontiguousarray(np.asarray(inputs[k], dtype=f))
    hW1, hW2, hW3, hW4 = g("hW1"), g("hW2"), g("hW3"), g("hW4")
    cW1 = g("cW1")

    wpack = np.zeros((P, NW), dtype=f)
    rpack = np.zeros((1, NR), dtype=f)
    def put(name, arr):
        c0, c1 = _COLS[name]
        arr = np.asarray(arr, dtype=f)
        wpack[:arr.shape[0], c0:c1] = arr
    def putr(name, vec):
        c0, c1 = _RCOLS[name]
        rpack[0, c0:c1] = np.asarray(vec, dtype=f).ravel()
    # layer-1 stationaries pre-scaled by 1/Nq: the chain's rhs are RAW
    # per-batch sums (reduced on the Pool engine straight into SBUF), and
    # tanh(W1^T S/Nq + b) == tanh(W1^T m + b)
    putr("w1a", hW1[0, :] / nq)
    putr("w1b", hW1[1, :] / nq)
    putr("cw1a", cW1[0, :] / nq)
    putr("cw1b", cW1[1, :] / nq)
    put("b1", g("hb1").reshape(128, 1))
    put("w2", hW2)
    put("b2", g("hb2").reshape(128, 1))
    put("w3", hW3)
    put("b3", g("hb3").reshape(64, 1))
    put("w4", hW4.reshape(64, 1))
    put("w4n", -hW4.reshape(64, 1))
    put("c2c", (hW3 @ hW4).reshape(128, 1))
    put("w1t", hW1.T)
    put("w2t", hW2.T)
    put("w3t", hW3.T)
    put("cb1", g("cb1").reshape(64, 1))
    put("cw2", g("cW2"))
    put("cb2", g("cb2").reshape(32, 1))
    put("cw3", g("cW3"))
    put("werr", g("cW3") @ np.ones((4, 1), dtype=f))
    rpack[0, _RCOLS["aux"][0]] = -0.5 * dt / nq
    rpack[0, _RCOLS["aux"][0] + 1] = dt / nq
    rpack[0, _RCOLS["aux"][0] + 2] = -dt / nq
    rpack[0, _RCOLS["aux"][0] + 3] = dt        # sum-space mq shift
    rpack[0, _RCOLS["aux"][0] + 4] = -dt       # sum-space mp shift

    in_maps = []
    for i in range(ncores):
        shard = state[i * bpc:(i + 1) * bpc].astype(bfloat16).reshape(
            2 * bpc, P, free)
        in_maps.append({"x": shard, "w": wpack, "wr": rpack})
    return in_maps


def kernel(**inputs):
    from concourse.bass_utils import run_bass_kernel_spmd

    if "nc" not in _CACHE:
        _CACHE["nc"] = build_nc()
    nc = _CACHE["nc"]
    in_maps = make_in_maps(inputs)
    res = run_bass_kernel_spmd(nc, in_maps, list(range(NCORES)))
    out = np.concatenate(
        [res.results[i]["y"].astype(np.float32).reshape(BPC, CH, H, W)
         for i in range(NCORES)],
        axis=0)
    return out


# revision 31
# speedup vs baseline: 1.0055x; 1.0028x over previous
"""Trainium2 Bass kernel for nn_HamiltonianDynamics.

Math: with q = state[:, :8], p = state[:, 8:], every MLP evaluation in the
reference operates on per-batch means of q/p. Adding a constant c to every
element of a [8,256,256] block shifts its mean by exactly c, so the whole
update collapses to per-batch stats:

  out = (state + off[b, half]) * scale
  off_q[b] = dt*gH[b,p]/Nq,  off_p[b] = -dt*gH[b,q]/Nq
  scale    = 1 - 0.1*err/(norm+1e-10)

Fully data-parallel SPMD, no collectives: each core owns 4 whole batches,
so the offsets (the only per-element-visible quantity) are exactly
computable locally. Approximations, each with bounded, documented error
far below the 2e-2 gate (the output error is dominated by bf16 I/O
quantization at ~1.7e-3 norm-relative):
  * I/O staged in bf16 (halves HBM traffic; keeps full relative precision
    on tiny elements unlike fp16 — wide exponent, no subnormal loss).
  * The three leapfrog gradient evaluations sit within O(dt*g/Nq) ~ 1e-7
    of the same point, so one backprop supplies both offsets (offset error
    ~1e-13 absolute, seven orders below the bf16 output ulp).
  * scale-1 is O(err/norm) ~ 1e-13, ten orders below bf16 resolution, so
    scale uses per-core unbiased estimates: local err mean; norm^2 from a
    2-tile sum-of-squares subsample (the off-dependent norm^2 correction
    terms, ~1e-11 relative, are dropped).

Pipeline per core (engine queues are in-order; emission order is tuned so
shadowable work never blocks the critical path):
  A. 17 chunked bf16 loads; per-chunk DVE sums via tensor_scalar+accum_out
     (bf16 4x mode) fold into PSUM means via 1/Nq-scaled ones-matmuls; ACT
     Square+accum sumsq on 2 tiles. The norm/sqrt/reciprocal path runs
     here too, pulling both ACT table loads off the critical path.
  B. After the last chunk: psum->sbuf means copy (q-half staged early),
     then one interleaved MLP forward+backward (casimir-old shadowed into
     its handoff gaps), fused junctions via scalar_tensor_tensor with the
     W3^T.W4 column host-folded, casimir-new, and a single matmul that
     fuses err reduction x recs scaling x partition broadcast (recs-filled
     stationary built during phase A).
  C. In-place bf16 transform y = x*scale + off (DVE 4x) + 17 chunked
     stores; the first store slice is 512 columns so it issues early.

Engine-AP constraint: compute-engine APs must start at partition 0, so all
per-batch row vectors are [1,nb] partition-0 rows and the 2-feature input
layers are two accumulated K=1 matmuls with [1,n] stationaries from a
single-descriptor row pack.
"""

import numpy as np
from ml_dtypes import bfloat16

NCORES = 8
B, CH, H, W = 32, 16, 256, 256
BPC = B // NCORES          # batches per core
NTILES = BPC * 2           # (batch, half) tiles per core
P = 128
FREE = (CH // 2) * H * W // P   # 4096
NQ = (CH // 2) * H * W          # 524288
NSSQ = 2                   # tiles subsampled for the norm estimate

# packed-weights layouts: tall [128, NW] (full-height tensors) and a row
# pack [1, NR] for the partition-0-only [1,n] stationaries — the row pack
# DMA is a single descriptor (~free) instead of n full-height columns
_COLS = {}
_RCOLS = {}


def _col_layout():
    c = 0
    def put(name, cols):
        nonlocal c
        _COLS[name] = (c, c + cols)
        c += cols
    put("b1", 1); put("w2", 128); put("b2", 1)
    put("w3", 64); put("b3", 1)
    put("w4", 1); put("w4n", 1); put("c2c", 1)
    put("w1t", 2); put("w2t", 128); put("w3t", 128)
    put("cb1", 1); put("cw2", 32); put("cb2", 1)
    put("cw3", 4); put("werr", 1)
    return c


def _row_layout():
    c = 0
    def put(name, cols):
        nonlocal c
        _RCOLS[name] = (c, c + cols)
        c += cols
    put("w1a", 128); put("w1b", 128)
    put("cw1a", 64); put("cw1b", 64)
    put("aux", 5)
    return c


NW = _col_layout()
NR = _row_layout()

_CACHE: dict = {}


def build_nc(ncores=NCORES, bpc=BPC, free=FREE):
    import concourse.bass as bass
    import concourse.bacc as bacc
    import concourse.tile as tile
    import concourse.mybir as mybir
    from contextlib import ExitStack

    f32 = mybir.dt.float32
    f16 = mybir.dt.bfloat16
    AL = mybir.AluOpType
    AF = mybir.ActivationFunctionType
    AX = mybir.AxisListType

    ntiles = bpc * 2
    nb = bpc
    nq = float(P * free)

    nc = bacc.Bacc("TRN2", target_bir_lowering=False, debug=False,
                   num_devices=ncores)

    x = nc.dram_tensor("x", [ntiles, P, free], f16, kind="ExternalInput").ap()
    w = nc.dram_tensor("w", [P, NW], f32, kind="ExternalInput").ap()
    wr = nc.dram_tensor("wr", [1, NR], f32, kind="ExternalInput").ap()
    y = nc.dram_tensor("y", [ntiles, P, free], f16, kind="ExternalOutput").ap()

    with tile.TileContext(nc) as tc, ExitStack() as ctx:
        xpool = ctx.enter_context(tc.tile_pool(name="xp", bufs=1))
        wpool = ctx.enter_context(tc.tile_pool(name="wp", bufs=1))
        scr = ctx.enter_context(tc.tile_pool(name="scr", bufs=2))
        ch = ctx.enter_context(tc.tile_pool(name="ch", bufs=2))
        keep = ctx.enter_context(tc.tile_pool(name="keep", bufs=1))
        psum = ctx.enter_context(tc.tile_pool(name="ps", bufs=4, space="PSUM"))
        pstat = ctx.enter_context(tc.tile_pool(name="pst", bufs=1, space="PSUM"))
        pcas = ctx.enter_context(tc.tile_pool(name="pcas", bufs=2, space="PSUM"))

        ones_col = wpool.tile([128, 1], f32)     # lhsT for partition sums
        nc.vector.memset(ones_col[:], 1.0)
        ones_bc = wpool.tile([1, 128], f32)      # lhsT for partition broadcast
        nc.vector.memset(ones_bc[:], 1.0)

        # ---- phase A: load shard + per-(batch,half) stats ----
        # Each tile loads as two half-chunks so the DVE sum accumulation
        # (tensor_scalar identity with accum_out, bf16 fast mode) trails the
        # DMA stream by only half a tile. The two halves' partition sums are
        # folded in PSUM via accumulated ones-matmuls. Sum-of-squares only on
        # the first NSSQ tiles via ACT Square+accum (norm estimate input).
        hf = free // 2
        part_ss = pstat.tile([1, NSSQ], f32, tag="sstat")
        # raw per-batch sums, s-major (cols 0:nb = q, nb:2nb = p), written
        # directly by per-tile Pool cross-partition reduces
        m_sb = keep.tile([1, ntiles], f32)
        xts = []
        for t in range(ntiles):
            xt = xpool.tile([P, free], f16, tag=f"x{t}")
            qf = hf // 2
            bounds = ([0, hf, 2 * hf] if t < ntiles - 1 else
                      [0, hf, 3 * qf, 3 * qf + 512, 4 * qf])
            st = keep.tile([128, len(bounds) - 1], f32, tag=f"st{t}")
            for c in range(len(bounds) - 1):
                sl = slice(bounds[c], bounds[c + 1])
                nc.sync.dma_start(xt[:, sl], x[t][:, sl])
                nc.vector.tensor_scalar(xt[:, sl], xt[:, sl], scalar1=1.0,
                                        scalar2=0.0, op0=AL.mult, op1=AL.add,
                                        accum_out=st[:, c:c + 1])
            # tile total via a Pool cross-partition reduce straight into the
            # sums row — no PE matmul, no PSUM->SBUF hop on the tail
            mc = (t % 2) * nb + t // 2
            nc.gpsimd.tensor_reduce(m_sb[0:1, mc:mc + 1], st[:],
                                    axis=AX.XYZWC, op=AL.add)
            if t < NSSQ:
                st2 = keep.tile([128, 1], f32, tag=f"ss{t}")
                sq = scr.tile([P, free], f16, tag=f"sq{t}")
                nc.scalar.activation(sq[:], xt[:], AF.Square,
                                     accum_out=st2[:, 0:1])
                nc.tensor.matmul(part_ss[0:1, t:t + 1], ones_col[:],
                                 st2[:, 0:1], start=True, stop=True)
            if t == NSSQ:
                # ---- early norm/scale-denominator path (runs during the
                # load phase). norm^2 = (ncores*ntiles/NSSQ)*(ssq subsample);
                # the off-dependent correction terms (2*off*sum + Nq*off^2)
                # are ~1e-11 of norm^2 — far below the subsample's own
                # statistical accuracy — and are dropped. Doing the sqrt
                # here keeps the ACT sqrt-table load (1.3us) off the
                # critical path: the tanh-set reload it forces also lands
                # before the chain starts.
                rs = keep.tile([1, NSSQ], f32)
                nc.vector.tensor_copy(rs[:], part_ss[:])
                norm2 = keep.tile([1, 1], f32)
                nc.vector.tensor_tensor(norm2[:], rs[0:1, 0:1], rs[0:1, 1:2],
                                        op=AL.add)
                nc.vector.tensor_scalar(norm2[:], norm2[:],
                                        scalar1=float(ncores * ntiles) / NSSQ,
                                        scalar2=None, op0=AL.mult)
                nrm = keep.tile([1, 1], f32)
                nc.scalar.sqrt(nrm[:], norm2[:])
                den = keep.tile([1, 1], f32)
                nc.vector.tensor_scalar(den[:], nrm[:], scalar1=1e-10,
                                        scalar2=None, op0=AL.add)
                rec = keep.tile([1, 1], f32)
                nc.vector.reciprocal(rec[:], den[:])
                recs = keep.tile([1, 1], f32)
                nc.vector.tensor_scalar(recs[:], rec[:],
                                        scalar1=-0.1 / (4.0 * nb),
                                        scalar2=None, op0=AL.mult)
                # rall[32,128] = recs everywhere: used as the stationary of
                # the final err matmul so reduction x recs-scale x partition-
                # broadcast collapse into that one matmul
                rrow = keep.tile([1, 128], f32)
                nc.vector.tensor_scalar(rrow[:], ones_bc[:],
                                        scalar1=recs[0:1, 0:1], scalar2=None,
                                        op0=AL.mult)
                prall = pcas.tile([32, 128], f32, tag="cps")
                nc.tensor.matmul(prall[:], ones_bc[0:1, 0:32], rrow[:],
                                 start=True, stop=True)
                rall = keep.tile([32, 128], f32)
                nc.vector.tensor_copy(rall[:], prall[:])
                # dummy tanh on the sqrt result: pulls the tanh-set table
                # reload (1.3us, forced by the sqrt-set switch above) into
                # the load phase. The data dependency on nrm stops the
                # out-of-order window from hoisting it before the sqrt.
                dummy = keep.tile([1, 1], f32)
                nc.scalar.activation(dummy[:], nrm[:], AF.Tanh)
            xts.append(xt)

        # packed weights: the single-descriptor row pack first (~free),
        # then the tall pack
        wrt = wpool.tile([1, NR], f32)
        nc.sync.dma_start(wrt[:], wr)
        wt = wpool.tile([P, NW], f32)
        nc.sync.dma_start(wt[:], w)

        def wap(name):
            if name in _RCOLS:
                c0, c1 = _RCOLS[name]
                return wrt[0:1, c0:c1]
            c0, c1 = _COLS[name]
            rows = {"b3": 64, "w4": 64, "w4n": 64, "c2c": 128, "w3t": 64,
                    "cb1": 64, "cw2": 64, "cb2": 32, "cw3": 32,
                    "werr": 32}.get(name, 128)
            return wt[0:rows, c0:c1]

        # raw per-batch sums, s-major (the 1/Nq lives in the layer-1
        # stationaries and the aux constants)
        mq = m_sb[0:1, 0:nb]
        mp = m_sb[0:1, nb:2 * nb]

        # ---- phase C: scalar chain (features on partitions, batch on free) --
        aux = wap("aux")
        aux1, aux2 = aux[0:1, 1:2], aux[0:1, 2:3]
        aux3, aux4 = aux[0:1, 3:4], aux[0:1, 4:5]

        # The three leapfrog gradient evaluations sit within O(dt*g/Nq)
        # ~ 1e-7 of the same point, so g1 == g2 == g3 to ~1e-6 relative and
        # one backprop supplies both offset rows:
        #   offq = dt*g[p]/Nq, offp = -dt*g[q]/Nq
        # (the collapse changes the offsets by ~1e-13 absolute — seven
        # orders below the bf16 output ulp).
        #
        # The casimir-at-original-means evaluation (g2o) is hand-interleaved
        # into the gH forward: every engine queue is in-order, so each g2o
        # op is emitted right after the gH op it can shadow.
        p1 = psum.tile([128, nb], f32, tag="ps")
        nc.tensor.matmul(p1[:], wap("w1a"), mq, start=True, stop=False)
        nc.tensor.matmul(p1[:], wap("w1b"), mp, start=False, stop=True)
        cq1 = pcas.tile([64, nb], f32, tag="cps")
        nc.tensor.matmul(cq1[:], wap("cw1a"), mq, start=True, stop=False)
        nc.tensor.matmul(cq1[:], wap("cw1b"), mp, start=False, stop=True)
        h1 = ch.tile([128, nb], f32, tag="h1")
        nc.scalar.activation(h1[:], p1[:], AF.Tanh, bias=wap("b1"))
        cg1 = ch.tile([64, nb], f32, tag="cg1")
        nc.scalar.activation(cg1[:], cq1[:], AF.Tanh, bias=wap("cb1"))
        p2 = psum.tile([128, nb], f32, tag="ps")
        nc.tensor.matmul(p2[:], wap("w2"), h1[:], start=True, stop=True)
        cq2 = pcas.tile([32, nb], f32, tag="cps")
        nc.tensor.matmul(cq2[:], wap("cw2"), cg1[:], start=True, stop=True)
        h2 = ch.tile([128, nb], f32, tag="h2")
        nc.scalar.activation(h2[:], p2[:], AF.Tanh, bias=wap("b2"))
        g2o = ch.tile([32, nb], f32, tag="g2o")
        nc.scalar.activation(g2o[:], cq2[:], AF.Tanh, bias=wap("cb2"))
        p3 = psum.tile([64, nb], f32, tag="ps")
        nc.tensor.matmul(p3[:], wap("w3"), h2[:], start=True, stop=True)
        h3 = ch.tile([64, nb], f32, tag="h3")
        nc.scalar.activation(h3[:], p3[:], AF.Tanh, bias=wap("b3"))
        # backward (see gH docstring for the d3/c2 folding)
        u3 = ch.tile([64, nb], f32, tag="d3")
        nc.vector.scalar_tensor_tensor(u3[:], h3[:], wap("w4n"), h3[:],
                                       op0=AL.mult, op1=AL.mult)
        pd2 = psum.tile([128, nb], f32, tag="ps")
        nc.tensor.matmul(pd2[:], wap("w3t"), u3[:], start=True, stop=True)
        t2 = ch.tile([128, nb], f32, tag="t2")
        nc.vector.tensor_tensor(t2[:], h2[:], h2[:], op=AL.mult)
        nc.vector.tensor_scalar(t2[:], t2[:], scalar1=-1.0, scalar2=1.0,
                                op0=AL.mult, op1=AL.add)
        d2 = ch.tile([128, nb], f32, tag="d2")
        nc.vector.scalar_tensor_tensor(d2[:], pd2[:], wap("c2c"), t2[:],
                                       op0=AL.add, op1=AL.mult)
        pd1 = psum.tile([128, nb], f32, tag="ps")
        nc.tensor.matmul(pd1[:], wap("w2t"), d2[:], start=True, stop=True)
        t1 = ch.tile([128, nb], f32, tag="t1")
        nc.vector.tensor_tensor(t1[:], h1[:], h1[:], op=AL.mult)
        nc.vector.tensor_scalar(t1[:], t1[:], scalar1=-1.0, scalar2=1.0,
                                op0=AL.mult, op1=AL.add)
        d1 = ch.tile([128, nb], f32, tag="d1")
        nc.vector.tensor_tensor(d1[:], t1[:], pd1[:], op=AL.mult)
        w1t = wap("w1t")
        pgq = psum.tile([1, nb], f32, tag="ps")
        nc.tensor.matmul(pgq[:], w1t[:, 0:1], d1[:], start=True, stop=True)
        pgp = psum.tile([1, nb], f32, tag="ps")
        nc.tensor.matmul(pgp[:], w1t[:, 1:2], d1[:], start=True, stop=True)
        g2ow = ch.tile([32, nb], f32, tag="g2ow")
        nc.vector.tensor_scalar(g2ow[:], g2o[:], scalar1=wap("werr"),
                                scalar2=None, op0=AL.mult)
        # shifted means via fused (pg * aux) + m — one DVE op each on the
        # g2n critical path; the raw offsets and their partition broadcast
        # run in parallel (they only gate the transform, which also needs
        # scale — the slower path)
        mpn = keep.tile([1, nb], f32)
        nc.vector.scalar_tensor_tensor(mpn[:], pgq[:], aux4, mp, op0=AL.mult,
                                       op1=AL.add)
        mq3 = keep.tile([1, nb], f32)
        nc.vector.scalar_tensor_tensor(mq3[:], pgp[:], aux3, mq, op0=AL.mult,
                                       op1=AL.add)

        # casimir err estimate at the shifted means. mpn is computed first
        # and consumed by the first accumulated matmul so the PE starts half
        # a hop sooner; the offset broadcast below is emitted after these
        # matmuls because it has ~2us of slack before the transform needs it
        cq1n = pcas.tile([64, nb], f32, tag="cps")
        nc.tensor.matmul(cq1n[:], wap("cw1b"), mpn[:], start=True, stop=False)
        nc.tensor.matmul(cq1n[:], wap("cw1a"), mq3[:], start=False, stop=True)

        Bv = keep.tile([1, 2 * nb], f32)
        nc.vector.tensor_scalar(Bv[0:1, 0:nb], pgp[:], scalar1=aux1,
                                scalar2=None, op0=AL.mult)
        nc.vector.tensor_scalar(Bv[0:1, nb:2 * nb], pgq[:], scalar1=aux2,
                                scalar2=None, op0=AL.mult)
        poffb = psum.tile([128, 2 * nb], f32, tag="ps")
        nc.tensor.matmul(poffb[:], ones_bc[:], Bv[:], start=True, stop=True)
        offb = keep.tile([128, 2 * nb], f32)
        nc.vector.tensor_copy(offb[:], poffb[:])

        cg1n = ch.tile([64, nb], f32, tag="cg1n")
        nc.scalar.activation(cg1n[:], cq1n[:], AF.Tanh, bias=wap("cb1"))
        cq2n = pcas.tile([32, nb], f32, tag="cps")
        nc.tensor.matmul(cq2n[:], wap("cw2"), cg1n[:], start=True, stop=True)
        g2n = ch.tile([32, nb], f32, tag="g2n")
        nc.scalar.activation(g2n[:], cq2n[:], AF.Tanh, bias=wap("cb2"))

        # err tail: errsum = sum(werr[j]*(g2n - g2o)[j,b]) with
        # werr = cW3 @ ones4 folded on the host; g2o*werr precomputed off
        # the critical path, so one fused DVE op + one matmul remain
        dws = keep.tile([32, 1], f32)
        dwt = ch.tile([32, nb], f32, tag="dwt")
        nc.vector.scalar_tensor_tensor(dwt[:], g2n[:], wap("werr"), g2ow[:],
                                       op0=AL.mult, op1=AL.subtract,
                                       accum_out=dws[:, 0:1])
        # scale-1 = recs * errsum on every partition in one matmul (rall is
        # the recs-filled stationary); the +1 rides the psum->sbuf copy
        pscale = psum.tile([128, 1], f32, tag="ps")
        nc.tensor.matmul(pscale[:], rall[:], dws[:], start=True, stop=True)
        scb = keep.tile([128, 1], f32)
        nc.vector.tensor_scalar(scb[:], pscale[:], scalar1=1.0, scalar2=None,
                                op0=AL.add)

        # ---- phase E: in-place transform + store (half tiles so the first
        # store launches half a tile after scale lands) ----
        # first two chunks sized so their transfer time covers the ~0.7us
        # per-DMA issue spacing (no DMA bubble while the store pipe fills);
        # issue alternates between the two HWDGE engines
        k = 0
        for t in range(ntiles):
            bl, h = t // 2, t % 2
            col = h * nb + bl
            xt = xts[t]
            bounds = [0, 1024, hf, 2 * hf] if t == 0 else [0, hf, 2 * hf]
            for c in range(len(bounds) - 1):
                sl = slice(bounds[c], bounds[c + 1])
                # y = x*scale + off (the off term is applied unscaled:
                # off*(1-scale) ~ 1e-20 — utterly below any representable
                # difference)
                nc.vector.tensor_scalar(xt[:, sl], xt[:, sl],
                                        scalar1=scb[:, 0:1],
                                        scalar2=offb[:, col:col + 1],
                                        op0=AL.mult, op1=AL.add)
                eng = nc.sync if k % 2 == 0 else nc.scalar
                eng.dma_start(y[t][:, sl], xt[:, sl])
                k += 1

    nc.compile()
    return nc


def make_in_maps(inputs, ncores=NCORES, bpc=BPC, free=FREE):
    state = np.asarray(inputs["state"])
    dt = float(np.asarray(inputs["dt"]))
    nq = float(P * free)
    f = np.float32
    g = lambda k: np.ascontiguousarray(np.asarray(inputs[k], dtype=f))
    hW1, hW2, hW3, hW4 = g("hW1"), g("hW2"), g("hW3"), g("hW4")
    cW1 = g("cW1")

    wpack = np.zeros((P, NW), dtype=f)
    rpack = np.zeros((1, NR), dtype=f)
    def put(name, arr):
        c0, c1 = _COLS[name]
        arr = np.asarray(arr, dtype=f)
        wpack[:arr.shape[0], c0:c1] = arr
    def putr(name, vec):
        c0, c1 = _RCOLS[name]
        rpack[0, c0:c1] = np.asarray(vec, dtype=f).ravel()
    # layer-1 stationaries pre-scaled by 1/Nq: the chain's rhs are RAW
    # per-batch sums (reduced on the Pool engine straight into SBUF), and
    # tanh(W1^T S/Nq + b) == tanh(W1^T m + b)
    putr("w1a", hW1[0, :] / nq)
    putr("w1b", hW1[1, :] / nq)
    putr("cw1a", cW1[0, :] / nq)
    putr("cw1b", cW1[1, :] / nq)
    put("b1", g("hb1").reshape(128, 1))
    put("w2", hW2)
    put("b2", g("hb2").reshape(128, 1))
    put("w3", hW3)
    put("b3", g("hb3").reshape(64, 1))
    put("w4", hW4.reshape(64, 1))
    put("w4n", -hW4.reshape(64, 1))
    put("c2c", (hW3 @ hW4).reshape(128, 1))
    put("w1t", hW1.T)
    put("w2t", hW2.T)
    put("w3t", hW3.T)
    put("cb1", g("cb1").reshape(64, 1))
    put("cw2", g("cW2"))
    put("cb2", g("cb2").reshape(32, 1))
    put("cw3", g("cW3"))
    put("werr", g("cW3") @ np.ones((4, 1), dtype=f))
    rpack[0, _RCOLS["aux"][0]] = -0.5 * dt / nq
    rpack[0, _RCOLS["aux"][0] + 1] = dt / nq
    rpack[0, _RCOLS["aux"][0] + 2] = -dt / nq
    rpack[0, _RCOLS["aux"][0] + 3] = dt        # sum-space mq shift
    rpack[0, _RCOLS["aux"][0] + 4] = -dt       # sum-space mp shift

    in_maps = []
    for i in range(ncores):
        shard = state[i * bpc:(i + 1) * bpc].astype(bfloat16).reshape(
            2 * bpc, P, free)
        in_maps.append({"x": shard, "w": wpack, "wr": rpack})
    return in_maps


def kernel(**inputs):
    from concourse.bass_utils import run_bass_kernel_spmd

    if "nc" not in _CACHE:
        _CACHE["nc"] = build_nc()
    nc = _CACHE["nc"]
    in_maps = make_in_maps(inputs)
    res = run_bass_kernel_spmd(nc, in_maps, list(range(NCORES)))
    out = np.concatenate(
        [res.results[i]["y"].astype(np.float32).reshape(BPC, CH, H, W)
         for i in range(NCORES)],
        axis=0)
    return out


# revision 35
# speedup vs baseline: 1.0215x; 1.0159x over previous
"""Trainium2 Bass kernel for nn_HamiltonianDynamics.

Math: with q = state[:, :8], p = state[:, 8:], every MLP evaluation in the
reference operates on per-batch means of q/p. Adding a constant c to every
element of a [8,256,256] block shifts its mean by exactly c, so the whole
update collapses to per-batch stats:

  out = (state + off[b, half]) * scale
  off_q[b] = dt*gH[b,p]/Nq,  off_p[b] = -dt*gH[b,q]/Nq
  scale    = 1 - 0.1*err/(norm+1e-10)

Fully data-parallel SPMD, no collectives: each core owns 4 whole batches,
so the offsets (the only per-element-visible quantity) are exactly
computable locally. Approximations, each with bounded, documented error
far below the 2e-2 gate (the output error is dominated by bf16 I/O
quantization at ~1.7e-3 norm-relative):
  * I/O staged in bf16 (halves HBM traffic; keeps full relative precision
    on tiny elements unlike fp16 — wide exponent, no subnormal loss).
  * The three leapfrog gradient evaluations sit within O(dt*g/Nq) ~ 1e-7
    of the same point, so one backprop supplies both offsets (offset error
    ~1e-13 absolute, seven orders below the bf16 output ulp).
  * scale-1 is O(err/norm) ~ 1e-13, ten orders below bf16 resolution, so
    scale uses per-core unbiased estimates: local err mean; norm^2 from a
    2-tile sum-of-squares subsample (the off-dependent norm^2 correction
    terms, ~1e-11 relative, are dropped).

Pipeline per core (engine queues are in-order; emission order is tuned so
shadowable work never blocks the critical path):
  A. 18 chunked bf16 loads; per-chunk DVE sums via tensor_scalar+accum_out
     (bf16 4x mode); each tile's total goes straight to the SBUF sums row
     via a Pool cross-partition reduce (no PE/PSUM hop — the 1/Nq lives in
     the host-scaled layer-1 stationaries). ACT Square+accum sumsq on 2
     tiles; the norm/sqrt/reciprocal path runs here too, pulling both ACT
     table loads off the critical path.
  B. After the last chunk: one interleaved MLP forward+backward
     (casimir-old shadowed into its handoff gaps), fused junctions via
     scalar_tensor_tensor with the W3^T.W4 column host-folded,
     casimir-new, and a single matmul that fuses err reduction x recs
     scaling x partition broadcast (recs-filled stationary built during
     phase A).
  C. In-place bf16 transform y = x*scale + off (DVE 4x) + chunked stores
     on alternating HWDGE engines; the first two chunks are sized so
     their transfer time covers the per-DMA issue spacing.

Engine-AP constraint: compute-engine APs must start at partition 0, so all
per-batch row vectors are [1,nb] partition-0 rows and the 2-feature input
layers are two accumulated K=1 matmuls with [1,n] stationaries from a
single-descriptor row pack.
"""

import numpy as np
from ml_dtypes import bfloat16

NCORES = 8
B, CH, H, W = 32, 16, 256, 256
BPC = B // NCORES          # batches per core
NTILES = BPC * 2           # (batch, half) tiles per core
P = 128
FREE = (CH // 2) * H * W // P   # 4096
NQ = (CH // 2) * H * W          # 524288
NSSQ = 2                   # tiles subsampled for the norm estimate

# packed-weights layouts: tall [128, NW] (full-height tensors) and a row
# pack [1, NR] for the partition-0-only [1,n] stationaries — the row pack
# DMA is a single descriptor (~free) instead of n full-height columns
_COLS = {}
_RCOLS = {}


def _col_layout():
    c = 0
    def put(name, cols):
        nonlocal c
        _COLS[name] = (c, c + cols)
        c += cols
    put("b1", 1); put("w2", 128); put("b2", 1)
    put("w3", 64); put("b3", 1)
    put("w4", 1); put("w4n", 1); put("c2c", 1)
    put("w1t", 2); put("w2t", 128); put("w3t", 128)
    put("cb1", 1); put("cw2", 32); put("cb2", 1)
    put("cw3", 4); put("werr", 1); put("cw2wn", 1); put("cw1t", 2)
    return c


def _row_layout():
    c = 0
    def put(name, cols):
        nonlocal c
        _RCOLS[name] = (c, c + cols)
        c += cols
    put("w1a", 128); put("w1b", 128)
    put("cw1a", 64); put("cw1b", 64)
    put("aux", 7)
    return c


NW = _col_layout()
NR = _row_layout()

_CACHE: dict = {}


def build_nc(ncores=NCORES, bpc=BPC, free=FREE):
    import concourse.bass as bass
    import concourse.bacc as bacc
    import concourse.tile as tile
    import concourse.mybir as mybir
    from contextlib import ExitStack

    f32 = mybir.dt.float32
    f16 = mybir.dt.bfloat16
    AL = mybir.AluOpType
    AF = mybir.ActivationFunctionType
    AX = mybir.AxisListType

    ntiles = bpc * 2
    nb = bpc
    nq = float(P * free)

    nc = bacc.Bacc("TRN2", target_bir_lowering=False, debug=False,
                   num_devices=ncores)

    x = nc.dram_tensor("x", [ntiles, P, free], f16, kind="ExternalInput").ap()
    w = nc.dram_tensor("w", [P, NW], f32, kind="ExternalInput").ap()
    wr = nc.dram_tensor("wr", [1, NR], f32, kind="ExternalInput").ap()
    y = nc.dram_tensor("y", [ntiles, P, free], f16, kind="ExternalOutput").ap()

    with tile.TileContext(nc) as tc, ExitStack() as ctx:
        xpool = ctx.enter_context(tc.tile_pool(name="xp", bufs=1))
        wpool = ctx.enter_context(tc.tile_pool(name="wp", bufs=1))
        scr = ctx.enter_context(tc.tile_pool(name="scr", bufs=2))
        ch = ctx.enter_context(tc.tile_pool(name="ch", bufs=2))
        keep = ctx.enter_context(tc.tile_pool(name="keep", bufs=1))
        psum = ctx.enter_context(tc.tile_pool(name="ps", bufs=4, space="PSUM"))
        pstat = ctx.enter_context(tc.tile_pool(name="pst", bufs=1, space="PSUM"))
        pcas = ctx.enter_context(tc.tile_pool(name="pcas", bufs=2, space="PSUM"))

        ones_col = wpool.tile([128, 1], f32)     # lhsT for partition sums
        nc.vector.memset(ones_col[:], 1.0)
        ones_bc = wpool.tile([1, 128], f32)      # lhsT for partition broadcast
        nc.vector.memset(ones_bc[:], 1.0)

        # ---- phase A: load shard + per-(batch,half) stats ----
        # Each tile loads as two half-chunks so the DVE sum accumulation
        # (tensor_scalar identity with accum_out, bf16 fast mode) trails the
        # DMA stream by only half a tile. The two halves' partition sums are
        # folded in PSUM via accumulated ones-matmuls. Sum-of-squares only on
        # the first NSSQ tiles via ACT Square+accum (norm estimate input).
        hf = free // 2
        part_ss = pstat.tile([1, NSSQ], f32, tag="sstat")
        # raw per-batch sums, s-major (cols 0:nb = q, nb:2nb = p), written
        # directly by per-tile Pool cross-partition reduces
        m_sb = keep.tile([1, ntiles], f32)
        xts = []
        for t in range(ntiles):
            xt = xpool.tile([P, free], f16, tag=f"x{t}")
            qf = hf // 2
            bounds = ([0, hf, 2 * hf] if t < ntiles - 1 else
                      [0, hf, 3 * qf, 3 * qf + 512, 4 * qf])
            st = keep.tile([128, len(bounds) - 1], f32, tag=f"st{t}")
            for c in range(len(bounds) - 1):
                sl = slice(bounds[c], bounds[c + 1])
                nc.sync.dma_start(xt[:, sl], x[t][:, sl])
                nc.vector.tensor_scalar(xt[:, sl], xt[:, sl], scalar1=1.0,
                                        scalar2=0.0, op0=AL.mult, op1=AL.add,
                                        accum_out=st[:, c:c + 1])
            # tile total via a Pool cross-partition reduce straight into the
            # sums row — no PE matmul, no PSUM->SBUF hop on the tail
            mc = (t % 2) * nb + t // 2
            nc.gpsimd.tensor_reduce(m_sb[0:1, mc:mc + 1], st[:],
                                    axis=AX.XYZWC, op=AL.add)
            if t < NSSQ:
                st2 = keep.tile([128, 1], f32, tag=f"ss{t}")
                sq = scr.tile([P, free], f16, tag=f"sq{t}")
                nc.scalar.activation(sq[:], xt[:], AF.Square,
                                     accum_out=st2[:, 0:1])
                nc.tensor.matmul(part_ss[0:1, t:t + 1], ones_col[:],
                                 st2[:, 0:1], start=True, stop=True)
            if t == NSSQ:
                # ---- early norm/scale-denominator path (runs during the
                # load phase). norm^2 = (ncores*ntiles/NSSQ)*(ssq subsample);
                # the off-dependent correction terms (2*off*sum + Nq*off^2)
                # are ~1e-11 of norm^2 — far below the subsample's own
                # statistical accuracy — and are dropped. Doing the sqrt
                # here keeps the ACT sqrt-table load (1.3us) off the
                # critical path: the tanh-set reload it forces also lands
                # before the chain starts.
                rs = keep.tile([1, NSSQ], f32)
                nc.vector.tensor_copy(rs[:], part_ss[:])
                norm2 = keep.tile([1, 1], f32)
                nc.vector.tensor_tensor(norm2[:], rs[0:1, 0:1], rs[0:1, 1:2],
                                        op=AL.add)
                nc.vector.tensor_scalar(norm2[:], norm2[:],
                                        scalar1=float(ncores * ntiles) / NSSQ,
                                        scalar2=None, op0=AL.mult)
                nrm = keep.tile([1, 1], f32)
                nc.scalar.sqrt(nrm[:], norm2[:])
                den = keep.tile([1, 1], f32)
                nc.vector.tensor_scalar(den[:], nrm[:], scalar1=1e-10,
                                        scalar2=None, op0=AL.add)
                rec = keep.tile([1, 1], f32)
                nc.vector.reciprocal(rec[:], den[:])
                recs = keep.tile([1, 1], f32)
                nc.vector.tensor_scalar(recs[:], rec[:],
                                        scalar1=-0.1 / (4.0 * nb),
                                        scalar2=None, op0=AL.mult)
                # rrow[1,128] = recs on every column: stationary of the
                # final scale matmul, fusing recs-scaling x partition
                # broadcast of the linearized err
                rrow = keep.tile([1, 128], f32)
                nc.vector.tensor_scalar(rrow[:], ones_bc[:],
                                        scalar1=recs[0:1, 0:1], scalar2=None,
                                        op0=AL.mult)
                # dummy tanh on the sqrt result: pulls the tanh-set table
                # reload (1.3us, forced by the sqrt-set switch above) into
                # the load phase. The data dependency on nrm stops the
                # out-of-order window from hoisting it before the sqrt.
                dummy = keep.tile([1, 1], f32)
                nc.scalar.activation(dummy[:], nrm[:], AF.Tanh)
            xts.append(xt)

        # packed weights: the single-descriptor row pack first (~free),
        # then the tall pack
        wrt = wpool.tile([1, NR], f32)
        nc.sync.dma_start(wrt[:], wr)
        wt = wpool.tile([P, NW], f32)
        nc.sync.dma_start(wt[:], w)

        def wap(name):
            if name in _RCOLS:
                c0, c1 = _RCOLS[name]
                return wrt[0:1, c0:c1]
            c0, c1 = _COLS[name]
            rows = {"b3": 64, "w4": 64, "w4n": 64, "c2c": 128, "w3t": 64,
                    "cb1": 64, "cw2": 64, "cb2": 32, "cw3": 32,
                    "werr": 32, "cw2wn": 64, "cw1t": 64}.get(name, 128)
            return wt[0:rows, c0:c1]

        # raw per-batch sums, s-major (the 1/Nq lives in the layer-1
        # stationaries and the aux constants)
        mq = m_sb[0:1, 0:nb]
        mp = m_sb[0:1, nb:2 * nb]

        # ---- phase C: scalar chain (features on partitions, batch on free) --
        aux = wap("aux")
        aux1, aux2 = aux[0:1, 1:2], aux[0:1, 2:3]
        aux3, aux4 = aux[0:1, 3:4], aux[0:1, 4:5]

        # The three leapfrog gradient evaluations sit within O(dt*g/Nq)
        # ~ 1e-7 of the same point, so g1 == g2 == g3 to ~1e-6 relative and
        # one backprop supplies both offset rows:
        #   offq = dt*g[p]/Nq, offp = -dt*g[q]/Nq
        # (the collapse changes the offsets by ~1e-13 absolute — seven
        # orders below the bf16 output ulp).
        #
        # The casimir-at-original-means evaluation (g2o) is hand-interleaved
        # into the gH forward: every engine queue is in-order, so each g2o
        # op is emitted right after the gH op it can shadow.
        p1 = psum.tile([128, nb], f32, tag="ps")
        nc.tensor.matmul(p1[:], wap("w1a"), mq, start=True, stop=False)
        nc.tensor.matmul(p1[:], wap("w1b"), mp, start=False, stop=True)
        cq1 = pcas.tile([64, nb], f32, tag="cps")
        nc.tensor.matmul(cq1[:], wap("cw1a"), mq, start=True, stop=False)
        nc.tensor.matmul(cq1[:], wap("cw1b"), mp, start=False, stop=True)
        h1 = ch.tile([128, nb], f32, tag="h1")
        nc.scalar.activation(h1[:], p1[:], AF.Tanh, bias=wap("b1"))
        cg1 = ch.tile([64, nb], f32, tag="cg1")
        nc.scalar.activation(cg1[:], cq1[:], AF.Tanh, bias=wap("cb1"))
        p2 = psum.tile([128, nb], f32, tag="ps")
        nc.tensor.matmul(p2[:], wap("w2"), h1[:], start=True, stop=True)
        h2 = ch.tile([128, nb], f32, tag="h2")
        nc.scalar.activation(h2[:], p2[:], AF.Tanh, bias=wap("b2"))
        # casimir Jacobian wrt (mq,mp) at the original means, shadowed under
        # the gH forward: J = cW1 @ [(1-cg1^2) o (cW2 cW3 ones4)], with the
        # constant part (cW1 cW2 cW3 ones4) host-folded into jc_q/jc_p
        uc = ch.tile([64, nb], f32, tag="uc")
        nc.vector.scalar_tensor_tensor(uc[:], cg1[:], wap("cw2wn"), cg1[:],
                                       op0=AL.mult, op1=AL.mult)
        cw1t = wap("cw1t")
        pjq = pcas.tile([1, nb], f32, tag="cps")
        nc.tensor.matmul(pjq[:], cw1t[:, 0:1], uc[:], start=True, stop=True)
        pjp = pcas.tile([1, nb], f32, tag="cps")
        nc.tensor.matmul(pjp[:], cw1t[:, 1:2], uc[:], start=True, stop=True)
        jq = keep.tile([1, nb], f32)
        nc.vector.tensor_scalar(jq[:], pjq[:], scalar1=1.0,
                                scalar2=aux[0:1, 5:6], op0=AL.mult,
                                op1=AL.add)
        jp = keep.tile([1, nb], f32)
        nc.vector.tensor_scalar(jp[:], pjp[:], scalar1=1.0,
                                scalar2=aux[0:1, 6:7], op0=AL.mult,
                                op1=AL.add)
        p3 = psum.tile([64, nb], f32, tag="ps")
        nc.tensor.matmul(p3[:], wap("w3"), h2[:], start=True, stop=True)
        h3 = ch.tile([64, nb], f32, tag="h3")
        nc.scalar.activation(h3[:], p3[:], AF.Tanh, bias=wap("b3"))
        # backward (see gH docstring for the d3/c2 folding)
        u3 = ch.tile([64, nb], f32, tag="d3")
        nc.vector.scalar_tensor_tensor(u3[:], h3[:], wap("w4n"), h3[:],
                                       op0=AL.mult, op1=AL.mult)
        pd2 = psum.tile([128, nb], f32, tag="ps")
        nc.tensor.matmul(pd2[:], wap("w3t"), u3[:], start=True, stop=True)
        t2 = ch.tile([128, nb], f32, tag="t2")
        nc.vector.tensor_tensor(t2[:], h2[:], h2[:], op=AL.mult)
        nc.vector.tensor_scalar(t2[:], t2[:], scalar1=-1.0, scalar2=1.0,
                                op0=AL.mult, op1=AL.add)
        d2 = ch.tile([128, nb], f32, tag="d2")
        nc.vector.scalar_tensor_tensor(d2[:], pd2[:], wap("c2c"), t2[:],
                                       op0=AL.add, op1=AL.mult)
        pd1 = psum.tile([128, nb], f32, tag="ps")
        nc.tensor.matmul(pd1[:], wap("w2t"), d2[:], start=True, stop=True)
        t1 = ch.tile([128, nb], f32, tag="t1")
        nc.vector.tensor_tensor(t1[:], h1[:], h1[:], op=AL.mult)
        nc.vector.tensor_scalar(t1[:], t1[:], scalar1=-1.0, scalar2=1.0,
                                op0=AL.mult, op1=AL.add)
        d1 = ch.tile([128, nb], f32, tag="d1")
        nc.vector.tensor_tensor(d1[:], t1[:], pd1[:], op=AL.mult)
        w1t = wap("w1t")
        pgq = psum.tile([1, nb], f32, tag="ps")
        nc.tensor.matmul(pgq[:], w1t[:, 0:1], d1[:], start=True, stop=True)
        pgp = psum.tile([1, nb], f32, tag="ps")
        nc.tensor.matmul(pgp[:], w1t[:, 1:2], d1[:], start=True, stop=True)
        # linearized casimir err: errsum = sum_b Jq(b)*offq(b)+Jp(b)*offp(b)
        # (the quadratic remainder is O(off^2) ~ 1e-6 relative on err, i.e.
        # ~1e-19 on scale). Three small DVE ops after the backward.
        e1 = keep.tile([1, nb], f32)
        nc.vector.scalar_tensor_tensor(e1[:], pgp[:], aux1, jq[:], op0=AL.mult,
                                       op1=AL.mult)
        e2 = keep.tile([1, nb], f32)
        nc.vector.scalar_tensor_tensor(e2[:], pgq[:], aux2, jp[:], op0=AL.mult,
                                       op1=AL.mult)
        esum = keep.tile([1, 1], f32)
        e12 = keep.tile([1, nb], f32)
        nc.vector.scalar_tensor_tensor(e12[:], e1[:], 1.0, e2[:],
                                       op0=AL.mult, op1=AL.add,
                                       accum_out=esum[:, 0:1])

        # raw offsets + partition broadcast for the transform
        Bv = keep.tile([1, 2 * nb], f32)
        nc.vector.tensor_scalar(Bv[0:1, 0:nb], pgp[:], scalar1=aux1,
                                scalar2=None, op0=AL.mult)
        nc.vector.tensor_scalar(Bv[0:1, nb:2 * nb], pgq[:], scalar1=aux2,
                                scalar2=None, op0=AL.mult)
        poffb = psum.tile([128, 2 * nb], f32, tag="ps")
        nc.tensor.matmul(poffb[:], ones_bc[:], Bv[:], start=True, stop=True)
        offb = keep.tile([128, 2 * nb], f32)
        nc.vector.tensor_copy(offb[:], poffb[:])
        # scale-1 = recs * errsum on every partition in one matmul (rrow is
        # the recs-filled stationary); the +1 rides the psum->sbuf copy
        pscale = psum.tile([128, 1], f32, tag="ps")
        nc.tensor.matmul(pscale[:], rrow[:], esum[:], start=True, stop=True)
        scb = keep.tile([128, 1], f32)
        nc.vector.tensor_scalar(scb[:], pscale[:], scalar1=1.0, scalar2=None,
                                op0=AL.add)

        # ---- phase E: in-place transform + store (half tiles so the first
        # store launches half a tile after scale lands) ----
        # first two chunks sized so their transfer time covers the ~0.7us
        # per-DMA issue spacing (no DMA bubble while the store pipe fills);
        # issue alternates between the two HWDGE engines
        k = 0
        for t in range(ntiles):
            bl, h = t // 2, t % 2
            col = h * nb + bl
            xt = xts[t]
            bounds = [0, 1024, hf, 2 * hf] if t == 0 else [0, hf, 2 * hf]
            for c in range(len(bounds) - 1):
                sl = slice(bounds[c], bounds[c + 1])
                # y = x*scale + off (the off term is applied unscaled:
                # off*(1-scale) ~ 1e-20 — utterly below any representable
                # difference)
                nc.vector.tensor_scalar(xt[:, sl], xt[:, sl],
                                        scalar1=scb[:, 0:1],
                                        scalar2=offb[:, col:col + 1],
                                        op0=AL.mult, op1=AL.add)
                eng = nc.sync if k % 2 == 0 else nc.scalar
                eng.dma_start(y[t][:, sl], xt[:, sl])
                k += 1

    nc.compile()
    return nc


def make_in_maps(inputs, ncores=NCORES, bpc=BPC, free=FREE):
    state = np.asarray(inputs["state"])
    dt = float(np.asarray(inputs["dt"]))
    nq = float(P * free)
    f = np.float32
    g = lambda k: np.ascontiguousarray(np.asarray(inputs[k], dtype=f))
    hW1, hW2, hW3, hW4 = g("hW1"), g("hW2"), g("hW3"), g("hW4")
    cW1 = g("cW1")

    wpack = np.zeros((P, NW), dtype=f)
    rpack = np.zeros((1, NR), dtype=f)
    def put(name, arr):
        c0, c1 = _COLS[name]
        arr = np.asarray(arr, dtype=f)
        wpack[:arr.shape[0], c0:c1] = arr
    def putr(name, vec):
        c0, c1 = _RCOLS[name]
        rpack[0, c0:c1] = np.asarray(vec, dtype=f).ravel()
    # layer-1 stationaries pre-scaled by 1/Nq: the chain's rhs are RAW
    # per-batch sums (reduced on the Pool engine straight into SBUF), and
    # tanh(W1^T S/Nq + b) == tanh(W1^T m + b)
    putr("w1a", hW1[0, :] / nq)
    putr("w1b", hW1[1, :] / nq)
    putr("cw1a", cW1[0, :] / nq)
    putr("cw1b", cW1[1, :] / nq)
    put("b1", g("hb1").reshape(128, 1))
    put("w2", hW2)
    put("b2", g("hb2").reshape(128, 1))
    put("w3", hW3)
    put("b3", g("hb3").reshape(64, 1))
    put("w4", hW4.reshape(64, 1))
    put("w4n", -hW4.reshape(64, 1))
    put("c2c", (hW3 @ hW4).reshape(128, 1))
    put("w1t", hW1.T)
    put("w2t", hW2.T)
    put("w3t", hW3.T)
    put("cb1", g("cb1").reshape(64, 1))
    put("cw2", g("cW2"))
    put("cb2", g("cb2").reshape(32, 1))
    put("cw3", g("cW3"))
    put("werr", g("cW3") @ np.ones((4, 1), dtype=f))
    cw2w = g("cW2") @ g("cW3") @ np.ones((4, 1), dtype=f)   # [64,1]
    put("cw2wn", -cw2w)
    put("cw1t", cW1.T)
    rpack[0, _RCOLS["aux"][0]] = -0.5 * dt / nq
    rpack[0, _RCOLS["aux"][0] + 1] = dt / nq
    rpack[0, _RCOLS["aux"][0] + 2] = -dt / nq
    rpack[0, _RCOLS["aux"][0] + 3] = dt        # sum-space mq shift
    rpack[0, _RCOLS["aux"][0] + 4] = -dt       # sum-space mp shift
    jc = cW1 @ cw2w                            # [2,1] const part of J
    rpack[0, _RCOLS["aux"][0] + 5] = float(jc[0, 0])
    rpack[0, _RCOLS["aux"][0] + 6] = float(jc[1, 0])

    in_maps = []
    for i in range(ncores):
        shard = state[i * bpc:(i + 1) * bpc].astype(bfloat16).reshape(
            2 * bpc, P, free)
        in_maps.append({"x": shard, "w": wpack, "wr": rpack})
    return in_maps


def kernel(**inputs):
    from concourse.bass_utils import run_bass_kernel_spmd

    if "nc" not in _CACHE:
        _CACHE["nc"] = build_nc()
    nc = _CACHE["nc"]
    in_maps = make_in_maps(inputs)
    res = run_bass_kernel_spmd(nc, in_maps, list(range(NCORES)))
    out = np.concatenate(
        [res.results[i]["y"].astype(np.float32).reshape(BPC, CH, H, W)
         for i in range(NCORES)],
        axis=0)
    return out


# revision 37
# speedup vs baseline: 1.0262x; 1.0046x over previous
"""Trainium2 Bass kernel for nn_HamiltonianDynamics.

Math: with q = state[:, :8], p = state[:, 8:], every MLP evaluation in the
reference operates on per-batch means of q/p. Adding a constant c to every
element of a [8,256,256] block shifts its mean by exactly c, so the whole
update collapses to per-batch stats:

  out = (state + off[b, half]) * scale
  off_q[b] = dt*gH[b,p]/Nq,  off_p[b] = -dt*gH[b,q]/Nq
  scale    = 1 - 0.1*err/(norm+1e-10)

Fully data-parallel SPMD, no collectives: each core owns 4 whole batches,
so the offsets (the only per-element-visible quantity) are exactly
computable locally. Approximations, each with bounded, documented error
far below the 2e-2 gate (the output error is dominated by bf16 I/O
quantization at ~1.7e-3 norm-relative):
  * I/O staged in bf16 (halves HBM traffic; keeps full relative precision
    on tiny elements unlike fp16 — wide exponent, no subnormal loss).
  * The three leapfrog gradient evaluations sit within O(dt*g/Nq) ~ 1e-7
    of the same point, so one backprop supplies both offsets (offset error
    ~1e-13 absolute, seven orders below the bf16 output ulp).
  * scale-1 is O(err/norm) ~ 1e-13, ten orders below bf16 resolution, so
    scale uses per-core unbiased estimates: local err mean; norm^2 from a
    2-tile sum-of-squares subsample (the off-dependent norm^2 correction
    terms, ~1e-11 relative, are dropped).
  * The casimir err is linearized at the original means:
    err = sum_b J(b).off(b) with the Jacobian J computed during the
    forward phase (quadratic remainder ~1e-6 relative on err, ~1e-19 on
    scale) — no shifted re-evaluation on the critical path.

Pipeline per core (engine queues are in-order; emission order is tuned so
shadowable work never blocks the critical path):
  A. 18 chunked bf16 loads; per-chunk DVE sums via tensor_scalar+accum_out
     (bf16 4x mode); each tile's total goes straight to the SBUF sums row
     via a Pool cross-partition reduce (no PE/PSUM hop — the 1/Nq lives in
     the host-scaled layer-1 stationaries). ACT Square+accum sumsq on 2
     tiles; the norm/sqrt/reciprocal path runs here too, pulling both ACT
     table loads off the critical path.
  B. After the last chunk: one interleaved MLP forward+backward (the
     casimir layer and its Jacobian shadowed into the handoff gaps),
     fused junctions via scalar_tensor_tensor with the W3^T.W4 column
     host-folded; after the backward only three [1,nb] DVE ops (the
     linearized err dot) and one matmul (err x recs x partition
     broadcast via a recs-filled row stationary) gate the transform.
  C. In-place bf16 transform y = x*scale + off (DVE 4x) + chunked stores
     on alternating HWDGE engines; the first two chunks are sized so
     their transfer time covers the per-DMA issue spacing.

Engine-AP constraint: compute-engine APs must start at partition 0, so all
per-batch row vectors are [1,nb] partition-0 rows and the 2-feature input
layers are two accumulated K=1 matmuls with [1,n] stationaries from a
single-descriptor row pack.
"""

import numpy as np
from ml_dtypes import bfloat16

NCORES = 8
B, CH, H, W = 32, 16, 256, 256
BPC = B // NCORES          # batches per core
NTILES = BPC * 2           # (batch, half) tiles per core
P = 128
FREE = (CH // 2) * H * W // P   # 4096
NQ = (CH // 2) * H * W          # 524288
NSSQ = 2                   # tiles subsampled for the norm estimate

# packed-weights layouts: tall [128, NW] (full-height tensors) and a row
# pack [1, NR] for the partition-0-only [1,n] stationaries — the row pack
# DMA is a single descriptor (~free) instead of n full-height columns
_COLS = {}
_RCOLS = {}


def _col_layout():
    c = 0
    def put(name, cols):
        nonlocal c
        _COLS[name] = (c, c + cols)
        c += cols
    put("b1", 1); put("w2", 128); put("b2", 1)
    put("w3", 64); put("b3", 1)
    put("w4", 1); put("w4n", 1); put("c2c", 1)
    put("w1t", 2); put("w2t", 128); put("w3t", 128)
    put("cb1", 1); put("cw2", 32); put("cb2", 1)
    put("cw3", 4); put("werr", 1); put("cw2wn", 1); put("cw1t", 2)
    return c


def _row_layout():
    c = 0
    def put(name, cols):
        nonlocal c
        _RCOLS[name] = (c, c + cols)
        c += cols
    put("w1a", 128); put("w1b", 128)
    put("cw1a", 64); put("cw1b", 64)
    put("aux", 7)
    return c


NW = _col_layout()
NR = _row_layout()

_CACHE: dict = {}


def build_nc(ncores=NCORES, bpc=BPC, free=FREE):
    import concourse.bass as bass
    import concourse.bacc as bacc
    import concourse.tile as tile
    import concourse.mybir as mybir
    from contextlib import ExitStack

    f32 = mybir.dt.float32
    f16 = mybir.dt.bfloat16
    AL = mybir.AluOpType
    AF = mybir.ActivationFunctionType
    AX = mybir.AxisListType

    ntiles = bpc * 2
    nb = bpc
    nq = float(P * free)

    nc = bacc.Bacc("TRN2", target_bir_lowering=False, debug=False,
                   num_devices=ncores)

    x = nc.dram_tensor("x", [ntiles, P, free], f16, kind="ExternalInput").ap()
    w = nc.dram_tensor("w", [P, NW], f32, kind="ExternalInput").ap()
    wr = nc.dram_tensor("wr", [1, NR], f32, kind="ExternalInput").ap()
    y = nc.dram_tensor("y", [ntiles, P, free], f16, kind="ExternalOutput").ap()

    with tile.TileContext(nc) as tc, ExitStack() as ctx:
        xpool = ctx.enter_context(tc.tile_pool(name="xp", bufs=1))
        wpool = ctx.enter_context(tc.tile_pool(name="wp", bufs=1))
        scr = ctx.enter_context(tc.tile_pool(name="scr", bufs=2))
        ch = ctx.enter_context(tc.tile_pool(name="ch", bufs=2))
        keep = ctx.enter_context(tc.tile_pool(name="keep", bufs=1))
        psum = ctx.enter_context(tc.tile_pool(name="ps", bufs=4, space="PSUM"))
        pstat = ctx.enter_context(tc.tile_pool(name="pst", bufs=1, space="PSUM"))
        pcas = ctx.enter_context(tc.tile_pool(name="pcas", bufs=2, space="PSUM"))

        ones_col = wpool.tile([128, 1], f32)     # lhsT for partition sums
        nc.vector.memset(ones_col[:], 1.0)
        ones_bc = wpool.tile([1, 128], f32)      # lhsT for partition broadcast
        nc.vector.memset(ones_bc[:], 1.0)
        one1 = wpool.tile([1, 1], f32)           # rhs for the +1 accumulate
        nc.vector.memset(one1[:], 1.0)

        # ---- phase A: load shard + per-(batch,half) stats ----
        # Each tile loads as two half-chunks so the DVE sum accumulation
        # (tensor_scalar identity with accum_out, bf16 fast mode) trails the
        # DMA stream by only half a tile. The two halves' partition sums are
        # folded in PSUM via accumulated ones-matmuls. Sum-of-squares only on
        # the first NSSQ tiles via ACT Square+accum (norm estimate input).
        hf = free // 2
        part_ss = pstat.tile([1, NSSQ], f32, tag="sstat")
        # raw per-batch sums, s-major (cols 0:nb = q, nb:2nb = p), written
        # directly by per-tile Pool cross-partition reduces
        m_sb = keep.tile([1, ntiles], f32)
        xts = []
        for t in range(ntiles):
            xt = xpool.tile([P, free], f16, tag=f"x{t}")
            qf = hf // 2
            bounds = ([0, hf, 2 * hf] if t < ntiles - 1 else
                      [0, hf, 3 * qf, 3 * qf + 512, 4 * qf])
            st = keep.tile([128, len(bounds) - 1], f32, tag=f"st{t}")
            for c in range(len(bounds) - 1):
                sl = slice(bounds[c], bounds[c + 1])
                nc.sync.dma_start(xt[:, sl], x[t][:, sl])
                nc.vector.tensor_scalar(xt[:, sl], xt[:, sl], scalar1=1.0,
                                        scalar2=0.0, op0=AL.mult, op1=AL.add,
                                        accum_out=st[:, c:c + 1])
            # tile total via a Pool cross-partition reduce straight into the
            # sums row — no PE matmul, no PSUM->SBUF hop on the tail
            mc = (t % 2) * nb + t // 2
            nc.gpsimd.tensor_reduce(m_sb[0:1, mc:mc + 1], st[:],
                                    axis=AX.XYZWC, op=AL.add)
            if t < NSSQ:
                st2 = keep.tile([128, 1], f32, tag=f"ss{t}")
                sq = scr.tile([P, free], f16, tag=f"sq{t}")
                nc.scalar.activation(sq[:], xt[:], AF.Square,
                                     accum_out=st2[:, 0:1])
                nc.tensor.matmul(part_ss[0:1, t:t + 1], ones_col[:],
                                 st2[:, 0:1], start=True, stop=True)
            if t == NSSQ:
                # ---- early norm/scale-denominator path (runs during the
                # load phase). norm^2 = (ncores*ntiles/NSSQ)*(ssq subsample);
                # the off-dependent correction terms (2*off*sum + Nq*off^2)
                # are ~1e-11 of norm^2 — far below the subsample's own
                # statistical accuracy — and are dropped. Doing the sqrt
                # here keeps the ACT sqrt-table load (1.3us) off the
                # critical path: the tanh-set reload it forces also lands
                # before the chain starts.
                rs = keep.tile([1, NSSQ], f32)
                nc.vector.tensor_copy(rs[:], part_ss[:])
                norm2 = keep.tile([1, 1], f32)
                nc.vector.tensor_tensor(norm2[:], rs[0:1, 0:1], rs[0:1, 1:2],
                                        op=AL.add)
                nc.vector.tensor_scalar(norm2[:], norm2[:],
                                        scalar1=float(ncores * ntiles) / NSSQ,
                                        scalar2=None, op0=AL.mult)
                nrm = keep.tile([1, 1], f32)
                nc.scalar.sqrt(nrm[:], norm2[:])
                den = keep.tile([1, 1], f32)
                nc.vector.tensor_scalar(den[:], nrm[:], scalar1=1e-10,
                                        scalar2=None, op0=AL.add)
                rec = keep.tile([1, 1], f32)
                nc.vector.reciprocal(rec[:], den[:])
                recs = keep.tile([1, 1], f32)
                nc.vector.tensor_scalar(recs[:], rec[:],
                                        scalar1=-0.1 / (4.0 * nb),
                                        scalar2=None, op0=AL.mult)
                # rrow[1,128] = recs on every column: stationary of the
                # final scale matmul, fusing recs-scaling x partition
                # broadcast of the linearized err
                rrow = keep.tile([1, 128], f32)
                nc.vector.tensor_scalar(rrow[:], ones_bc[:],
                                        scalar1=recs[0:1, 0:1], scalar2=None,
                                        op0=AL.mult)
                # dummy tanh on the sqrt result: pulls the tanh-set table
                # reload (1.3us, forced by the sqrt-set switch above) into
                # the load phase. The data dependency on nrm stops the
                # out-of-order window from hoisting it before the sqrt.
                dummy = keep.tile([1, 1], f32)
                nc.scalar.activation(dummy[:], nrm[:], AF.Tanh)
            xts.append(xt)

        # packed weights: the single-descriptor row pack first (~free),
        # then the tall pack
        wrt = wpool.tile([1, NR], f32)
        nc.sync.dma_start(wrt[:], wr)
        wt = wpool.tile([P, NW], f32)
        nc.sync.dma_start(wt[:], w)

        def wap(name):
            if name in _RCOLS:
                c0, c1 = _RCOLS[name]
                return wrt[0:1, c0:c1]
            c0, c1 = _COLS[name]
            rows = {"b3": 64, "w4": 64, "w4n": 64, "c2c": 128, "w3t": 64,
                    "cb1": 64, "cw2": 64, "cb2": 32, "cw3": 32,
                    "werr": 32, "cw2wn": 64, "cw1t": 64}.get(name, 128)
            return wt[0:rows, c0:c1]

        # raw per-batch sums, s-major (the 1/Nq lives in the layer-1
        # stationaries and the aux constants)
        mq = m_sb[0:1, 0:nb]
        mp = m_sb[0:1, nb:2 * nb]

        # ---- phase C: scalar chain (features on partitions, batch on free) --
        aux = wap("aux")
        aux1, aux2 = aux[0:1, 1:2], aux[0:1, 2:3]
        aux3, aux4 = aux[0:1, 3:4], aux[0:1, 4:5]

        # The three leapfrog gradient evaluations sit within O(dt*g/Nq)
        # ~ 1e-7 of the same point, so g1 == g2 == g3 to ~1e-6 relative and
        # one backprop supplies both offset rows:
        #   offq = dt*g[p]/Nq, offp = -dt*g[q]/Nq
        # (the collapse changes the offsets by ~1e-13 absolute — seven
        # orders below the bf16 output ulp).
        #
        # The casimir-at-original-means evaluation (g2o) is hand-interleaved
        # into the gH forward: every engine queue is in-order, so each g2o
        # op is emitted right after the gH op it can shadow.
        p1 = psum.tile([128, nb], f32, tag="ps")
        nc.tensor.matmul(p1[:], wap("w1a"), mq, start=True, stop=False)
        nc.tensor.matmul(p1[:], wap("w1b"), mp, start=False, stop=True)
        cq1 = pcas.tile([64, nb], f32, tag="cps")
        nc.tensor.matmul(cq1[:], wap("cw1a"), mq, start=True, stop=False)
        nc.tensor.matmul(cq1[:], wap("cw1b"), mp, start=False, stop=True)
        h1 = ch.tile([128, nb], f32, tag="h1")
        nc.scalar.activation(h1[:], p1[:], AF.Tanh, bias=wap("b1"))
        cg1 = ch.tile([64, nb], f32, tag="cg1")
        nc.scalar.activation(cg1[:], cq1[:], AF.Tanh, bias=wap("cb1"))
        p2 = psum.tile([128, nb], f32, tag="ps")
        nc.tensor.matmul(p2[:], wap("w2"), h1[:], start=True, stop=True)
        h2 = ch.tile([128, nb], f32, tag="h2")
        nc.scalar.activation(h2[:], p2[:], AF.Tanh, bias=wap("b2"))
        # casimir Jacobian wrt (mq,mp) at the original means, shadowed under
        # the gH forward: J = cW1 @ [(1-cg1^2) o (cW2 cW3 ones4)], with the
        # constant part (cW1 cW2 cW3 ones4) host-folded into jc_q/jc_p
        uc = ch.tile([64, nb], f32, tag="uc")
        nc.vector.scalar_tensor_tensor(uc[:], cg1[:], wap("cw2wn"), cg1[:],
                                       op0=AL.mult, op1=AL.mult)
        cw1t = wap("cw1t")
        pjq = pcas.tile([1, nb], f32, tag="cps")
        nc.tensor.matmul(pjq[:], cw1t[:, 0:1], uc[:], start=True, stop=True)
        pjp = pcas.tile([1, nb], f32, tag="cps")
        nc.tensor.matmul(pjp[:], cw1t[:, 1:2], uc[:], start=True, stop=True)
        jq = keep.tile([1, nb], f32)
        nc.vector.tensor_scalar(jq[:], pjq[:], scalar1=1.0,
                                scalar2=aux[0:1, 5:6], op0=AL.mult,
                                op1=AL.add)
        jp = keep.tile([1, nb], f32)
        nc.vector.tensor_scalar(jp[:], pjp[:], scalar1=1.0,
                                scalar2=aux[0:1, 6:7], op0=AL.mult,
                                op1=AL.add)
        p3 = psum.tile([64, nb], f32, tag="ps")
        nc.tensor.matmul(p3[:], wap("w3"), h2[:], start=True, stop=True)
        h3 = ch.tile([64, nb], f32, tag="h3")
        nc.scalar.activation(h3[:], p3[:], AF.Tanh, bias=wap("b3"))
        # backward (see gH docstring for the d3/c2 folding)
        u3 = ch.tile([64, nb], f32, tag="d3")
        nc.vector.scalar_tensor_tensor(u3[:], h3[:], wap("w4n"), h3[:],
                                       op0=AL.mult, op1=AL.mult)
        pd2 = psum.tile([128, nb], f32, tag="ps")
        nc.tensor.matmul(pd2[:], wap("w3t"), u3[:], start=True, stop=True)
        t2 = ch.tile([128, nb], f32, tag="t2")
        nc.vector.tensor_tensor(t2[:], h2[:], h2[:], op=AL.mult)
        nc.vector.tensor_scalar(t2[:], t2[:], scalar1=-1.0, scalar2=1.0,
                                op0=AL.mult, op1=AL.add)
        d2 = ch.tile([128, nb], f32, tag="d2")
        nc.vector.scalar_tensor_tensor(d2[:], pd2[:], wap("c2c"), t2[:],
                                       op0=AL.add, op1=AL.mult)
        pd1 = psum.tile([128, nb], f32, tag="ps")
        nc.tensor.matmul(pd1[:], wap("w2t"), d2[:], start=True, stop=True)
        t1 = ch.tile([128, nb], f32, tag="t1")
        nc.vector.tensor_tensor(t1[:], h1[:], h1[:], op=AL.mult)
        nc.vector.tensor_scalar(t1[:], t1[:], scalar1=-1.0, scalar2=1.0,
                                op0=AL.mult, op1=AL.add)
        d1 = ch.tile([128, nb], f32, tag="d1")
        nc.vector.tensor_tensor(d1[:], t1[:], pd1[:], op=AL.mult)
        w1t = wap("w1t")
        pgq = psum.tile([1, nb], f32, tag="ps")
        nc.tensor.matmul(pgq[:], w1t[:, 0:1], d1[:], start=True, stop=True)
        pgp = psum.tile([1, nb], f32, tag="ps")
        nc.tensor.matmul(pgp[:], w1t[:, 1:2], d1[:], start=True, stop=True)
        # linearized casimir err: errsum = sum_b Jq(b)*offq(b)+Jp(b)*offp(b)
        # (the quadratic remainder is O(off^2) ~ 1e-6 relative on err, i.e.
        # ~1e-19 on scale). Three small DVE ops after the backward.
        e1 = keep.tile([1, nb], f32)
        nc.vector.scalar_tensor_tensor(e1[:], pgp[:], aux1, jq[:], op0=AL.mult,
                                       op1=AL.mult)
        e2 = keep.tile([1, nb], f32)
        nc.vector.scalar_tensor_tensor(e2[:], pgq[:], aux2, jp[:], op0=AL.mult,
                                       op1=AL.mult)
        esum = keep.tile([1, 1], f32)
        e12 = keep.tile([1, nb], f32)
        nc.vector.scalar_tensor_tensor(e12[:], e1[:], 1.0, e2[:],
                                       op0=AL.mult, op1=AL.add,
                                       accum_out=esum[:, 0:1])

        # raw offsets + partition broadcast for the transform
        Bv = keep.tile([1, 2 * nb], f32)
        nc.vector.tensor_scalar(Bv[0:1, 0:nb], pgp[:], scalar1=aux1,
                                scalar2=None, op0=AL.mult)
        nc.vector.tensor_scalar(Bv[0:1, nb:2 * nb], pgq[:], scalar1=aux2,
                                scalar2=None, op0=AL.mult)
        poffb = psum.tile([128, 2 * nb], f32, tag="ps")
        nc.tensor.matmul(poffb[:], ones_bc[:], Bv[:], start=True, stop=True)
        # scale on every partition in one accumulated matmul pair: rrow
        # (recs-filled stationary) x errsum, +1 via ones x 1. The transform
        # reads it straight from PSUM (scalar operands don't break the DVE
        # fast mode), so no psum->sbuf hop remains before the stores.
        pscale = psum.tile([128, 1], f32, tag="ps")
        nc.tensor.matmul(pscale[:], rrow[:], esum[:], start=True, stop=False)
        nc.tensor.matmul(pscale[:], ones_bc[:], one1[:], start=False,
                         stop=True)

        # ---- phase E: in-place transform + store (half tiles so the first
        # store launches half a tile after scale lands) ----
        # first two chunks sized so their transfer time covers the ~0.7us
        # per-DMA issue spacing (no DMA bubble while the store pipe fills);
        # issue alternates between the two HWDGE engines
        k = 0
        for t in range(ntiles):
            bl, h = t // 2, t % 2
            col = h * nb + bl
            xt = xts[t]
            bounds = [0, 1024, hf, 2 * hf] if t == 0 else [0, hf, 2 * hf]
            for c in range(len(bounds) - 1):
                sl = slice(bounds[c], bounds[c + 1])
                # y = x*scale + off (the off term is applied unscaled:
                # off*(1-scale) ~ 1e-20 — utterly below any representable
                # difference)
                nc.vector.tensor_scalar(xt[:, sl], xt[:, sl],
                                        scalar1=pscale[:, 0:1],
                                        scalar2=poffb[:, col:col + 1],
                                        op0=AL.mult, op1=AL.add)
                eng = nc.sync if k % 2 == 0 else nc.scalar
                eng.dma_start(y[t][:, sl], xt[:, sl])
                k += 1

    nc.compile()
    return nc


def make_in_maps(inputs, ncores=NCORES, bpc=BPC, free=FREE):
    state = np.asarray(inputs["state"])
    dt = float(np.asarray(inputs["dt"]))
    nq = float(P * free)
    f = np.float32
    g = lambda k: np.ascontiguousarray(np.asarray(inputs[k], dtype=f))
    hW1, hW2, hW3, hW4 = g("hW1"), g("hW2"), g("hW3"), g("hW4")
    cW1 = g("cW1")

    wpack = np.zeros((P, NW), dtype=f)
    rpack = np.zeros((1, NR), dtype=f)
    def put(name, arr):
        c0, c1 = _COLS[name]
        arr = np.asarray(arr, dtype=f)
        wpack[:arr.shape[0], c0:c1] = arr
    def putr(name, vec):
        c0, c1 = _RCOLS[name]
        rpack[0, c0:c1] = np.asarray(vec, dtype=f).ravel()
    # layer-1 stationaries pre-scaled by 1/Nq: the chain's rhs are RAW
    # per-batch sums (reduced on the Pool engine straight into SBUF), and
    # tanh(W1^T S/Nq + b) == tanh(W1^T m + b)
    putr("w1a", hW1[0, :] / nq)
    putr("w1b", hW1[1, :] / nq)
    putr("cw1a", cW1[0, :] / nq)
    putr("cw1b", cW1[1, :] / nq)
    put("b1", g("hb1").reshape(128, 1))
    put("w2", hW2)
    put("b2", g("hb2").reshape(128, 1))
    put("w3", hW3)
    put("b3", g("hb3").reshape(64, 1))
    put("w4", hW4.reshape(64, 1))
    put("w4n", -hW4.reshape(64, 1))
    put("c2c", (hW3 @ hW4).reshape(128, 1))
    put("w1t", hW1.T)
    put("w2t", hW2.T)
    put("w3t", hW3.T)
    put("cb1", g("cb1").reshape(64, 1))
    put("cw2", g("cW2"))
    put("cb2", g("cb2").reshape(32, 1))
    put("cw3", g("cW3"))
    put("werr", g("cW3") @ np.ones((4, 1), dtype=f))
    cw2w = g("cW2") @ g("cW3") @ np.ones((4, 1), dtype=f)   # [64,1]
    put("cw2wn", -cw2w)
    put("cw1t", cW1.T)
    rpack[0, _RCOLS["aux"][0]] = -0.5 * dt / nq
    rpack[0, _RCOLS["aux"][0] + 1] = dt / nq
    rpack[0, _RCOLS["aux"][0] + 2] = -dt / nq
    rpack[0, _RCOLS["aux"][0] + 3] = dt        # sum-space mq shift
    rpack[0, _RCOLS["aux"][0] + 4] = -dt       # sum-space mp shift
    jc = cW1 @ cw2w                            # [2,1] const part of J
    rpack[0, _RCOLS["aux"][0] + 5] = float(jc[0, 0])
    rpack[0, _RCOLS["aux"][0] + 6] = float(jc[1, 0])

    in_maps = []
    for i in range(ncores):
        shard = state[i * bpc:(i + 1) * bpc].astype(bfloat16).reshape(
            2 * bpc, P, free)
        in_maps.append({"x": shard, "w": wpack, "wr": rpack})
    return in_maps


def kernel(**inputs):
    from concourse.bass_utils import run_bass_kernel_spmd

    if "nc" not in _CACHE:
        _CACHE["nc"] = build_nc()
    nc = _CACHE["nc"]
    in_maps = make_in_maps(inputs)
    res = run_bass_kernel_spmd(nc, in_maps, list(range(NCORES)))
    out = np.concatenate(
        [res.results[i]["y"].astype(np.float32).reshape(BPC, CH, H, W)
         for i in range(NCORES)],
        axis=0)
    return out
